# revision 9
# baseline (speedup 1.0000x reference)
"""Distributed Trainium2 kernel for nn_AddAttention_154618823089.

Computation (see reference):
    q = rope(bf16(hidden @ Wq.T)); k = rope(bf16(hidden @ Wk.T))
    o[b,l] = sum_{j<=l} exp(q_l . k_j / sqrt(DIM))          (no softmax norm)
    out = relu(o @ fc1_w.T + fc1_b) @ fc2_w.T + fc2_b

Sharding: every core c handles the strided row set {r : r % 8 == c} of
BOTH batches (512 rows each).  Striding makes the causal workload identical
on every core; 8-rank AllGathers ride the fast RDH algorithm (~162 GB/s).

v3 structure (vs the 2x4MB-gather baseline):
  - the k exchange is FOUR pipelined 8-rank AllGathers: (b0,kb0-2),
    (b1,kb0-2), (b0,kb3), (b1,kb3).  Score compute chases the gathers:
    s<=2 blocks of a batch only need kb0-2, so 32 of 40 score blocks (and
    3/4 of the MLP + output DMA) run while later gathers are in flight.
    The ncfw startup barrier (~22->55us, runtime-fixed) gates the first
    op either way; splitting costs ~2us/op on the cc stream but moves
    most compute off the tail.
  - kt tiles [128, t4, kb, r8, jj128] per (batch, phase, t-half); per-rank
    dmas with contiguous >=1KB source lines, spread over sync/scalar/gpsimd.
  - PE warmer chain (vector-paced dummy matmuls) covers the AG wait so
    the HAM clock gate keeps the PE at 2.4GHz into the scores phase.
  - bf16 output (cast to f32 on host), halved cos/sin loads (both batches
    share the same strided row positions).
  - fp8 DoubleRow matmuls everywhere; exp fused with row-sum via accum_out.
"""

import sys
import types

import numpy as np
from ml_dtypes import bfloat16, float8_e4m3

import concourse.bacc as bacc
import concourse.bass as bass
import concourse.mybir as mybir
import concourse.tile as tile
from concourse.bass_utils import run_bass_kernel_spmd


def _install_ntff_hook():
    """The container's antenv lacks axon_hooks; provide it so trace=True can
    capture NTFF profiles (exec_time_ns) through the axon PJRT library."""
    if "antenv.axon_hooks" in sys.modules:
        return
    try:
        sys.path.insert(0, "/root/.axon_site/trn_agent_boot")
        import trn_boot

        mod = types.ModuleType("antenv.axon_hooks")
        _h = {"hook": None}
        mod.set_axon_ntff_profile_hook = lambda h: _h.__setitem__("hook", h)
        mod.get_axon_ntff_profile_hook = lambda: _h["hook"]
        sys.modules["antenv.axon_hooks"] = mod
        import antenv

        antenv.axon_hooks = mod
        mod.set_axon_ntff_profile_hook(
            trn_boot._ntff_profile_via_ctypes("/opt/axon/libaxon_pjrt.so"))
    except Exception:
        pass


_install_ntff_hook()

B, L, DIM, INNER = 2, 4096, 1024, 16
ROPE_BASE = 32.0
NCORES = 8
RB = L // NCORES       # rows per core per batch (512)
RLOC = 2 * RB          # local q/k rows per core (both batches, 1024)
NSUB = RB // 128       # q subtiles per core per batch (4)
NDT = DIM // 128       # d tiles (8)
NDP = NDT // 2         # DoubleRow d-tile pairs (4)
KA = 3                 # kb blocks in gather phase A (kb 0..2)
SCALE = 1.0 / float(np.sqrt(DIM))
MASK_NEG = -1.0e6
CHUNK = 3              # psum banks per score chunk
NWARM_GAP = 40         # PE warmers during the AG wait
NWARM_MID = 4          # between scores_A(b0) and scores_A(b1)
NWARM_B = 8            # between scores_B(b0) and scores_B(b1)
F32 = mybir.dt.float32
BF16 = mybir.dt.bfloat16
F8 = mybir.dt.float8e4
DR = mybir.MatmulPerfMode.DoubleRow

_NC_CACHE = {}


def _build_nc():
    nc = bacc.Bacc("TRN2", target_bir_lowering=False, debug=False,
                   num_devices=NCORES, num_swdge_queues=4)

    hT = nc.dram_tensor("hT", [DIM, RLOC], F8, kind="ExternalInput")
    wqT = nc.dram_tensor("wqT", [DIM, DIM], F8, kind="ExternalInput")
    wkT = nc.dram_tensor("wkT", [DIM, DIM], F8, kind="ExternalInput")
    cosh = nc.dram_tensor("cosh", [DIM // 2, RB], BF16, kind="ExternalInput")
    sinh = nc.dram_tensor("sinh", [DIM // 2, RB], BF16, kind="ExternalInput")
    mask0 = nc.dram_tensor("mask0", [128, 512], F32, kind="ExternalInput")
    mask1 = nc.dram_tensor("mask1", [128, 512], F32, kind="ExternalInput")
    w1b_d = nc.dram_tensor("w1b", [128, 32], F32, kind="ExternalInput")
    b1b_d = nc.dram_tensor("b1b", [128, 32], F32, kind="ExternalInput")
    w2aug = nc.dram_tensor("w2aug", [INNER + 1, DIM], BF16, kind="ExternalInput")
    onesrow = nc.dram_tensor("onesrow", [1, RB], BF16, kind="ExternalInput")
    out_d = nc.dram_tensor("out", [RLOC, DIM], BF16, kind="ExternalOutput")

    # split bounce/gather: phase A = kb 0..2 (384 cols), phase B = kb 3
    bncA = [nc.dram_tensor(f"bncA{b}", [128, NDT, 128 * KA], F8)
            for b in range(B)]
    bncB = [nc.dram_tensor(f"bncB{b}", [128, NDT, 128], F8)
            for b in range(B)]
    GA = [nc.dram_tensor(f"GA{b}", [NCORES * 128, NDT, 128 * KA], F8,
                         addr_space="Shared") for b in range(B)]
    GB = [nc.dram_tensor(f"GB{b}", [NCORES * 128, NDT, 128], F8,
                         addr_space="Shared") for b in range(B)]

    groups = [list(range(NCORES))]

    with tile.TileContext(nc) as tc:
        with (
            tc.tile_pool(name="big", bufs=1) as big,
            tc.tile_pool(name="tmp", bufs=2) as tmp,
            tc.tile_pool(name="stg", bufs=2) as stg,
            tc.tile_pool(name="rsp", bufs=1) as rsp,
            tc.tile_pool(name="obp", bufs=4) as obp,
            tc.tile_pool(name="ps", bufs=7, space="PSUM") as pps,
            tc.tile_pool(name="po", bufs=1, space="PSUM") as ppo,
        ):
            # ---- inputs -> SBUF as DoubleRow pair tiles, spread on queues --
            h_r = hT.rearrange("(dp k2 p) r -> dp p k2 r", dp=NDP, k2=2, p=128)
            wk_r = wkT.rearrange("(dp k2 p) r -> dp p k2 r",
                                 dp=NDP, k2=2, p=128)
            wq_r = wqT.rearrange("(dp k2 p) r -> dp p k2 r",
                                 dp=NDP, k2=2, p=128)
            h_t, wk_t, wq_t = [], [], []
            for dp in range(NDP):
                th = big.tile([128, 2, RLOC], F8, tag=f"h{dp}", name=f"h{dp}")
                nc.sync.dma_start(th[:], h_r[dp])
                h_t.append(th)
                tw = big.tile([128, 2, DIM], F8, tag=f"wk{dp}", name=f"wk{dp}")
                nc.scalar.dma_start(tw[:], wk_r[dp])
                wk_t.append(tw)
            cos_t, sin_t = [], []
            for ci in range(NDT // 2):
                tc_ = big.tile([128, RB], BF16, tag=f"cos{ci}",
                               name=f"cos{ci}")
                nc.sync.dma_start(tc_[:], cosh[128 * ci:128 * (ci + 1), :])
                cos_t.append(tc_)
                ts_ = big.tile([128, RB], BF16, tag=f"sin{ci}",
                               name=f"sin{ci}")
                nc.scalar.dma_start(ts_[:], sinh[128 * ci:128 * (ci + 1), :])
                sin_t.append(ts_)
            for dp in range(NDP):
                tw = big.tile([128, 2, DIM], F8, tag=f"wq{dp}", name=f"wq{dp}")
                nc.sync.dma_start(tw[:], wq_r[dp])
                wq_t.append(tw)
            mask_sb = [big.tile([128, 512], F32, tag=f"mask{h}",
                                name=f"mask_sb{h}") for h in range(2)]
            nc.gpsimd.dma_start(mask_sb[0][:], mask0[:])
            nc.gpsimd.dma_start(mask_sb[1][:], mask1[:])
            w1b_sb = big.tile([128, 32], F32, tag="w1b")
            nc.gpsimd.dma_start(w1b_sb[:], w1b_d[:])
            b1b_sb = big.tile([128, 32], F32, tag="b1b")
            nc.gpsimd.dma_start(b1b_sb[:], b1b_d[:])
            w2_sb = big.tile([INNER + 1, DIM], BF16, tag="w2")
            nc.gpsimd.dma_start(w2_sb[:], w2aug[:])
            z_aug = big.tile([INNER + 1, RB], BF16, tag="zaug")
            nc.gpsimd.dma_start(z_aug[INNER:INNER + 1, :], onesrow[:])

            # pre-load the Exp activation table off the critical path
            etab = tmp.tile([1, 4], F32, tag="etab", name="etab")
            nc.scalar.activation(etab[:], w1b_sb[0:1, 0:4],
                                 mybir.ActivationFunctionType.Exp)

            def project_half(w_t, proj, rt, bounce=False):
                """proj[:, :, 512rt:512rt+512] = fp8(rope(W @ h^T)).
                DoubleRow fp8 matmuls -> psum f32 -> bf16 staging (scalar)
                -> rope on vector -> fp8 slots (dt, dt+4); do-order
                interleaves the (dt, dt+4) halves so RoPE pairs complete
                (and optionally bounce to DRAM) right behind PE."""
                cols = slice(512 * rt, 512 * (rt + 1))
                pbf = stg.tile([128, NDT, 512], BF16, tag="pbf",
                               name=f"pbf{rt}")

                def rope_pair(dt):
                    # both batches share the same strided row positions, so
                    # the same cos/sin tiles serve rt=0 and rt=1
                    cm = cos_t[dt][:, :]
                    sm = sin_t[dt][:, :]
                    lo = pbf[:, dt, :]
                    hi = pbf[:, dt + NDT // 2, :]
                    ta = tmp.tile([128, 512], BF16, tag="ta", name="ta")
                    tb = tmp.tile([128, 512], BF16, tag="tb", name="tb")
                    td = tmp.tile([128, 512], BF16, tag="td", name="td")
                    nc.vector.tensor_mul(ta[:], lo, cm)
                    nc.vector.tensor_mul(tb[:], lo, sm)
                    nc.vector.tensor_mul(td[:], hi, sm)
                    nc.vector.tensor_sub(proj[:, dt, cols], ta[:], td[:])
                    nc.vector.tensor_mul(ta[:], hi, cm)
                    nc.vector.tensor_add(proj[:, dt + NDT // 2, cols],
                                         ta[:], tb[:])
                    if bounce:
                        # both rope slots of the pair, split at the kb0-2 /
                        # kb3 boundary for the phased gathers
                        eng = nc.sync if dt % 2 else nc.scalar
                        ca = slice(512 * rt, 512 * rt + 128 * KA)
                        cb = slice(512 * rt + 128 * KA, 512 * (rt + 1))
                        eng.dma_start(bncA[rt][:, dt::NDT // 2, :],
                                      proj[:, dt::NDT // 2, ca])
                        eng.dma_start(bncB[rt][:, dt::NDT // 2, :],
                                      proj[:, dt::NDT // 2, cb])

                order = [x for pair in zip(range(NDT // 2),
                                           range(NDT // 2, NDT))
                         for x in pair]            # 0,4,1,5,2,6,3,7
                for do in order:
                    ps = pps.tile([128, 512], F32, tag="ps",
                                  name=f"psp{rt}{do}")
                    for dp in range(NDP):
                        nc.tensor.matmul(
                            ps[:], w_t[dp][:, :, 128 * do:128 * (do + 1)],
                            h_t[dp][:, :, cols],
                            start=(dp == 0), stop=(dp == NDP - 1),
                            perf_mode=DR,
                        )
                    nc.scalar.activation(pbf[:, do, :], ps[:],
                                         mybir.ActivationFunctionType.Copy)
                    if do >= NDT // 2:
                        rope_pair(do - NDT // 2)

            # ---- k per batch: project+rope+bounce, then the four phased
            # all-gathers back to back on the cc stream; q projects during
            # the collectives ------------------------------------------------
            k_rope = big.tile([128, NDT, RLOC], F8, tag="krope")
            project_half(wk_t, k_rope, 0, bounce=True)
            nc.gpsimd.collective_compute(
                "AllGather", mybir.AluOpType.bypass, replica_groups=groups,
                ins=[bncA[0].ap().opt()], outs=[GA[0].ap().opt()])
            project_half(wk_t, k_rope, 1, bounce=True)
            nc.gpsimd.collective_compute(
                "AllGather", mybir.AluOpType.bypass, replica_groups=groups,
                ins=[bncA[1].ap().opt()], outs=[GA[1].ap().opt()])
            nc.gpsimd.collective_compute(
                "AllGather", mybir.AluOpType.bypass, replica_groups=groups,
                ins=[bncB[0].ap().opt()], outs=[GB[0].ap().opt()])
            nc.gpsimd.collective_compute(
                "AllGather", mybir.AluOpType.bypass, replica_groups=groups,
                ins=[bncB[1].ap().opt()], outs=[GB[1].ap().opt()])

            # ---- q: project + rope (overlaps with the collectives) ----
            q_rope = big.tile([128, NDT, RLOC], F8, tag="qrope")
            project_half(wq_t, q_rope, 0)
            project_half(wq_t, q_rope, 1)

            # ---- PE warmers: vector-paced dummy matmuls keep the HAM
            # clock gate open across the AG wait ----------------------------
            wsb = big.tile([128, 512], BF16, tag="wsb")
            nc.vector.tensor_copy(wsb[:], cos_t[0][:])

            def warmers(n, pfx):
                for i in range(n):
                    po = ppo.tile([128, 512], F32, tag="po",
                                  name=f"warm{pfx}{i}")
                    nc.tensor.matmul(po[:], cos_t[0][:, 0:128], wsb[:],
                                     start=True, stop=True)
                    nc.vector.tensor_copy(wsb[:], po[:])

            warmers(NWARM_GAP, "g")

            # ---- gathered-K -> SBUF ----------------------------------------
            gA_r = [GA[b].rearrange("(r p) t (kb jj) -> r p t kb jj",
                                    r=NCORES, p=128, kb=KA, jj=128)
                    for b in range(B)]
            gB_r = [GB[b].rearrange("(r p) t jj -> r p t jj",
                                    r=NCORES, p=128)
                    for b in range(B)]
            _kteng = [nc.sync, nc.scalar, nc.gpsimd]

            def load_ktA(b):
                # ktA layout: [128 (d in tile), t4, kb3, r8, jj128] with a
                # SEPARATE tile per t-half, so score matmuls on dp 0-1
                # start as soon as the first half of the transfer lands
                kts, i = [], b
                for dh in range(2):
                    kt = big.tile([128, NDT // 2, KA, NCORES, 128], F8,
                                  tag=f"ktA{b}{dh}", name=f"ktA{b}{dh}")
                    for r in range(NCORES):
                        eng = _kteng[i % 3]
                        i += 1
                        eng.dma_start(
                            kt[:, :, :, r, :],
                            gA_r[b][r, :, 4 * dh:4 * (dh + 1), :, :])
                    kts.append(kt)
                return kts

            def load_ktB(b):
                kt = big.tile([128, NDT, NCORES, 128], F8,
                              tag=f"ktB{b}", name=f"ktB{b}")
                for r in range(NCORES):
                    eng = _kteng[(b + r) % 3]
                    eng.dma_start(kt[:, :, r, :], gB_r[b][r])
                return kt

            ktA = [load_ktA(b) for b in range(B)]
            ktB = [load_ktB(b) for b in range(B)]

            o_sb = big.tile([128, B * NSUB], F32, tag="o")
            rs_all = {}

            def score_chunk(s, blist, rs_t, rhs_fn):
                for c0 in range(0, len(blist), CHUNK):
                    chunk = blist[c0:c0 + CHUNK]
                    psl = [pps.tile([128, 512], F32, tag="ps",
                                    name=f"ps{s}{c0}_{i}")
                           for i in range(len(chunk))]
                    for dp in range(NDP):
                        lhsT_s = None
                        for (b, kb, hh), ps in zip(chunk, psl):
                            if lhsT_s is None:
                                lhsT_s = q_rope[:, 2 * dp:2 * dp + 2,
                                                RB * b + 128 * s:
                                                RB * b + 128 * (s + 1)]
                            nc.tensor.matmul(
                                ps[:], lhsT_s, rhs_fn(b, kb, hh, dp),
                                start=(dp == 0), stop=(dp == NDP - 1),
                                perf_mode=DR,
                            )
                    for (b, kb, hh), ps in zip(chunk, psl):
                        if kb == s:
                            nc.vector.tensor_add(ps[:], ps[:],
                                                 mask_sb[hh][:])
                        nc.scalar.activation(
                            ps[:], ps[:],
                            mybir.ActivationFunctionType.Exp,
                            scale=SCALE,
                            accum_out=rs_t[:, 2 * kb + hh:2 * kb + hh + 1],
                        )

            def rhs_A(b, kb, hh, dp):
                dpl = 2 * (dp % 2)
                return ktA[b][dp // 2][:, dpl:dpl + 2, kb,
                                       4 * hh:4 * (hh + 1), :]

            def rhs_B(b, kb, hh, dp):
                return ktB[b][:, 2 * dp:2 * dp + 2,
                              4 * hh:4 * (hh + 1), :]

            def scores_A(b):
                # kb 0..2 blocks: everything for s<=2, plus s=3's kb 0..2
                for s in range(NSUB):
                    rs_t = rsp.tile([128, 2 * NSUB], F32, tag=f"rs{b}{s}",
                                    name=f"rs{b}{s}")
                    rs_all[(b, s)] = rs_t
                    blist = [(b, kb, hh) for kb in range(min(s + 1, KA))
                             for hh in range(2)]
                    score_chunk(s, blist, rs_t, rhs_A)
                for s in range(KA):
                    nc.vector.reduce_sum(
                        o_sb[:, NSUB * b + s:NSUB * b + s + 1],
                        rs_all[(b, s)][:, 0:2 * (s + 1)],
                        axis=mybir.AxisListType.X)
                    mlp_sub(b, s)

            def scores_B(b):
                s = NSUB - 1
                rs_t = rs_all[(b, s)]
                blist = [(b, KA, hh) for hh in range(2)]
                score_chunk(s, blist, rs_t, rhs_B)
                nc.vector.reduce_sum(
                    o_sb[:, NSUB * b + s:NSUB * b + s + 1],
                    rs_t[:, 0:2 * NSUB], axis=mybir.AxisListType.X)
                mlp_sub(b, s)

            def mlp_sub(b, s):
                # o_sb[p, b*NSUB+s] is local row b*RB + 128s + p.
                # z[row, n] = relu(o[row]*w1[n] + b1[n]) with o as a
                # per-partition scalar, DVE-transposed into z_aug[n, row],
                # then out rows = z_aug.T @ w2aug.
                col = NSUB * b + s
                zrow = tmp.tile([128, 32], F32, tag="zr", name=f"zr{b}{s}")
                nc.vector.tensor_scalar_mul(zrow[:], w1b_sb[:],
                                            o_sb[:, col:col + 1])
                nc.vector.tensor_add(zrow[:], zrow[:], b1b_sb[:])
                zrb = tmp.tile([128, 32], BF16, tag="zrb",
                               name=f"zrb{b}{s}")
                nc.vector.tensor_scalar_max(zrb[:], zrow[:], 0.0)
                zts = tmp.tile([32, 128], BF16, tag="zts", name=f"zts{b}{s}")
                for g in range(4):
                    nc.vector.transpose(zts[0:32, 32 * g:32 * (g + 1)],
                                        zrb[32 * g:32 * (g + 1), :])
                nc.vector.tensor_copy(z_aug[0:INNER, 128 * s:128 * (s + 1)],
                                      zts[0:INNER, :])
                row0 = RB * b + 128 * s
                ob = obp.tile([128, DIM], BF16, tag="ob", name=f"ob{b}{s}")
                for hh in range(2):
                    po = ppo.tile([128, 512], F32, tag="po",
                                  name=f"po{b}{s}{hh}")
                    nc.tensor.matmul(po[:],
                                     z_aug[:, 128 * s:128 * (s + 1)],
                                     w2_sb[:, 512 * hh:512 * (hh + 1)],
                                     start=True, stop=True)
                    nc.vector.tensor_copy(ob[:, 512 * hh:512 * (hh + 1)],
                                          po[:])
                eng = nc.gpsimd if s % 2 else nc.sync
                eng.dma_start(out_d[row0:row0 + 128, :], ob[:])

            scores_A(0)
            warmers(NWARM_MID, "m")
            scores_A(1)
            scores_B(0)
            warmers(NWARM_B, "b")
            scores_B(1)

    nc.compile()
    return nc


def get_nc():
    if "nc" not in _NC_CACHE:
        _NC_CACHE["nc"] = _build_nc()
    return _NC_CACHE["nc"]


def make_in_maps(hidden_states, Wq, Wk, fc1_w, fc1_b, fc2_w, fc2_b):
    hidden_states = np.asarray(hidden_states, dtype=np.float32)
    Wq = np.asarray(Wq, dtype=np.float32)
    Wk = np.asarray(Wk, dtype=np.float32)
    fc1_w = np.asarray(fc1_w, dtype=np.float32)
    fc1_b = np.asarray(fc1_b, dtype=np.float32)
    fc2_w = np.asarray(fc2_w, dtype=np.float32)
    fc2_b = np.asarray(fc2_b, dtype=np.float32)

    wqT = np.ascontiguousarray(Wq.T).astype(float8_e4m3)
    wkT = np.ascontiguousarray(Wk.T).astype(float8_e4m3)
    w1b = np.zeros((128, 32), dtype=np.float32)
    w1b[:, 0:INNER] = fc1_w.reshape(1, INNER)
    b1b = np.zeros((128, 32), dtype=np.float32)
    b1b[:, 0:INNER] = fc1_b.reshape(1, INNER)
    w2aug = np.concatenate([fc2_w.T, fc2_b[None, :]], axis=0).astype(bfloat16)

    inv_freq = ROPE_BASE ** (-np.arange(0, DIM, 2, dtype=np.float32) / DIM)

    in_maps = []
    for c in range(NCORES):
        rows = np.arange(RB) * NCORES + c            # global rows, per batch
        hT = np.concatenate(
            [hidden_states[b, rows, :].T for b in range(B)],
            axis=1).astype(float8_e4m3)              # [DIM, RLOC]
        ang = rows[:, None].astype(np.float32) * inv_freq[None, :]  # [RB,512]
        cosh = np.ascontiguousarray(np.cos(ang).T).astype(bfloat16)
        sinh = np.ascontiguousarray(np.sin(ang).T).astype(bfloat16)
        # mask_h[p, (jc-4h)*128+t]: allow k col (rank jc, t) for q row p iff
        # 8t + jc <= 8p + c  (boundary subtile; same for every s and batch)
        p = np.arange(128)[:, None, None]
        t = np.arange(128)[None, None, :]
        masks = []
        for h in range(2):
            jc = (np.arange(4) + 4 * h)[None, :, None]
            allow = (NCORES * t + jc) <= (NCORES * p + c)
            masks.append(np.where(allow, 0.0, MASK_NEG)
                         .astype(np.float32).reshape(128, 512))
        in_maps.append({
            "hT": np.ascontiguousarray(hT),
            "wqT": wqT, "wkT": wkT,
            "cosh": cosh, "sinh": sinh,
            "mask0": masks[0], "mask1": masks[1],
            "w1b": w1b, "b1b": b1b, "w2aug": w2aug,
            "onesrow": np.ones((1, RB), dtype=bfloat16),
        })
    return in_maps


def assemble_output(results):
    out = np.empty((B, L, DIM), dtype=np.float32)
    for c in range(NCORES):
        for b in range(B):
            out[b, c::NCORES, :] = (
                results[c]["out"][RB * b:RB * (b + 1)].astype(np.float32))
    return out


def run(trace=False, **inputs):
    nc = get_nc()
    in_maps = make_in_maps(**inputs)
    res = run_bass_kernel_spmd(nc, in_maps, core_ids=list(range(NCORES)),
                               trace=trace)
    return assemble_output(res.results), res


def kernel(**inputs) -> np.ndarray:
    out, _ = run(trace=False, **inputs)
    return out


# revision 10
# speedup vs baseline: 1.1039x; 1.1039x over previous
"""Distributed Trainium2 kernel for nn_AddAttention_154618823089.

Computation (see reference):
    q = rope(bf16(hidden @ Wq.T)); k = rope(bf16(hidden @ Wk.T))
    o[b,l] = sum_{j<=l} exp(q_l . k_j / sqrt(DIM))          (no softmax norm)
    out = relu(o @ fc1_w.T + fc1_b) @ fc2_w.T + fc2_b

Sharding: every core c handles the strided row set {r : r % 8 == c} of
BOTH batches (512 rows each).  Striding makes the causal workload identical
on every core, and taking rows from both batches makes the k exchange a
single fast 8-rank RDH AllGather per batch (4-rank groups fall back to the
slow Mesh path; finer splits pay a ~20us per-op cc cost).

v4 (over the 202us baseline):
  - fp8 DoubleRow pipeline as before (projections, score matmuls); exp
    fused with row-sum via accum_out; MLP per subtile
  - RoPE sin-muls moved to gpsimd so the vector queue stops lagging the
    PE by ~16us at the end of the projection phase
  - PE warmer chain (vector-paced dummy matmuls) spans the AG0 wait and
    the scores(0)->scores(1) gap so the HAM clock gate keeps the PE at
    2.4GHz instead of 1.2GHz through the scores phases
  - bf16 output (cast to f32 on host): halves output HBM traffic that
    competes with AllGather1
  - cos/sin loads halved (both batches share the same strided rows)
  - Exp activation table preloaded off the critical path
  - batch-1 kt loads stay OFF the scalar queue (scores(0) exps would be
    head-of-line blocked behind their AG1-gated DMAs)
"""

import sys
import types

import numpy as np
from ml_dtypes import bfloat16, float8_e4m3

import concourse.bacc as bacc
import concourse.bass as bass
import concourse.mybir as mybir
import concourse.tile as tile
from concourse.bass_utils import run_bass_kernel_spmd


def _install_ntff_hook():
    """The container's antenv lacks axon_hooks; provide it so trace=True can
    capture NTFF profiles (exec_time_ns) through the axon PJRT library."""
    if "antenv.axon_hooks" in sys.modules:
        return
    try:
        sys.path.insert(0, "/root/.axon_site/trn_agent_boot")
        import trn_boot

        mod = types.ModuleType("antenv.axon_hooks")
        _h = {"hook": None}
        mod.set_axon_ntff_profile_hook = lambda h: _h.__setitem__("hook", h)
        mod.get_axon_ntff_profile_hook = lambda: _h["hook"]
        sys.modules["antenv.axon_hooks"] = mod
        import antenv

        antenv.axon_hooks = mod
        mod.set_axon_ntff_profile_hook(
            trn_boot._ntff_profile_via_ctypes("/opt/axon/libaxon_pjrt.so"))
    except Exception:
        pass


_install_ntff_hook()

B, L, DIM, INNER = 2, 4096, 1024, 16
ROPE_BASE = 32.0
NCORES = 8
RB = L // NCORES       # rows per core per batch (512)
RLOC = 2 * RB          # local q/k rows per core (both batches, 1024)
NSUB = RB // 128       # q subtiles per core per batch (4)
NDT = DIM // 128       # d tiles (8)
NDP = NDT // 2         # DoubleRow d-tile pairs (4)
SCALE = 1.0 / float(np.sqrt(DIM))
MASK_NEG = -1.0e6
CHUNK = 3              # psum banks per score chunk
NWARM_GAP = 36         # PE warmers across the AG0 wait
NWARM_MID = 12         # PE warmers across the scores(0)->scores(1) gap
F32 = mybir.dt.float32
BF16 = mybir.dt.bfloat16
F8 = mybir.dt.float8e4
DR = mybir.MatmulPerfMode.DoubleRow

_NC_CACHE = {}


def _build_nc():
    nc = bacc.Bacc("TRN2", target_bir_lowering=False, debug=False,
                   num_devices=NCORES, num_swdge_queues=4)

    hT = nc.dram_tensor("hT", [DIM, RLOC], F8, kind="ExternalInput")
    wqT = nc.dram_tensor("wqT", [DIM, DIM], F8, kind="ExternalInput")
    wkT = nc.dram_tensor("wkT", [DIM, DIM], F8, kind="ExternalInput")
    cosh = nc.dram_tensor("cosh", [DIM // 2, RB], BF16, kind="ExternalInput")
    sinh = nc.dram_tensor("sinh", [DIM // 2, RB], BF16, kind="ExternalInput")
    mask0 = nc.dram_tensor("mask0", [128, 512], F32, kind="ExternalInput")
    mask1 = nc.dram_tensor("mask1", [128, 512], F32, kind="ExternalInput")
    w1b_d = nc.dram_tensor("w1b", [128, 32], F32, kind="ExternalInput")
    b1b_d = nc.dram_tensor("b1b", [128, 32], F32, kind="ExternalInput")
    w2aug = nc.dram_tensor("w2aug", [INNER + 1, DIM], BF16, kind="ExternalInput")
    onesrow = nc.dram_tensor("onesrow", [1, RB], BF16, kind="ExternalInput")
    out_d = nc.dram_tensor("out", [RLOC, DIM], BF16, kind="ExternalOutput")

    # one bounce + AllGather per batch: small collectives pay a ~20us
    # fixed per-op cost on the cc stream, so two 4MB-out gathers beat any
    # finer split
    kb_bounce = [nc.dram_tensor(f"kTb{b}", [128, NDT, RB], F8)
                 for b in range(B)]
    G = [nc.dram_tensor(f"G{b}", [NCORES * 128, NDT, RB], F8,
                        addr_space="Shared") for b in range(B)]

    groups = [list(range(NCORES))]

    with tile.TileContext(nc) as tc:
        with (
            tc.tile_pool(name="big", bufs=1) as big,
            tc.tile_pool(name="tmp", bufs=2) as tmp,
            tc.tile_pool(name="stg", bufs=2) as stg,
            tc.tile_pool(name="rsp", bufs=2) as rsp,
            tc.tile_pool(name="obp", bufs=4) as obp,
            tc.tile_pool(name="ps", bufs=7, space="PSUM") as pps,
            tc.tile_pool(name="po", bufs=1, space="PSUM") as ppo,
        ):
            # ---- inputs -> SBUF as DoubleRow pair tiles, spread on queues --
            # (dp p k2 r) views land each pair tile in ONE dma each
            h_r = hT.rearrange("(dp k2 p) r -> dp p k2 r", dp=NDP, k2=2, p=128)
            wk_r = wkT.rearrange("(dp k2 p) r -> dp p k2 r",
                                 dp=NDP, k2=2, p=128)
            wq_r = wqT.rearrange("(dp k2 p) r -> dp p k2 r",
                                 dp=NDP, k2=2, p=128)
            h_t, wk_t, wq_t = [], [], []
            for dp in range(NDP):
                th = big.tile([128, 2, RLOC], F8, tag=f"h{dp}", name=f"h{dp}")
                nc.sync.dma_start(th[:], h_r[dp])
                h_t.append(th)
                tw = big.tile([128, 2, DIM], F8, tag=f"wk{dp}", name=f"wk{dp}")
                nc.scalar.dma_start(tw[:], wk_r[dp])
                wk_t.append(tw)
            cos_t, sin_t = [], []
            for ci in range(NDT // 2):
                tc_ = big.tile([128, RB], BF16, tag=f"cos{ci}",
                               name=f"cos{ci}")
                nc.sync.dma_start(tc_[:], cosh[128 * ci:128 * (ci + 1), :])
                cos_t.append(tc_)
                ts_ = big.tile([128, RB], BF16, tag=f"sin{ci}",
                               name=f"sin{ci}")
                nc.scalar.dma_start(ts_[:], sinh[128 * ci:128 * (ci + 1), :])
                sin_t.append(ts_)
            for dp in range(NDP):
                # wq reuses wk's slots (k projection is done by then)
                tw = big.tile([128, 2, DIM], F8, tag=f"wk{dp}", name=f"wq{dp}")
                nc.sync.dma_start(tw[:], wq_r[dp])
                wq_t.append(tw)
            mask_sb = [big.tile([128, 512], F32, tag=f"mask{h}",
                                name=f"mask_sb{h}") for h in range(2)]
            nc.gpsimd.dma_start(mask_sb[0][:], mask0[:])
            nc.gpsimd.dma_start(mask_sb[1][:], mask1[:])
            w1b_sb = big.tile([128, 32], F32, tag="w1b")
            nc.gpsimd.dma_start(w1b_sb[:], w1b_d[:])
            b1b_sb = big.tile([128, 32], F32, tag="b1b")
            nc.gpsimd.dma_start(b1b_sb[:], b1b_d[:])
            w2_sb = big.tile([INNER + 1, DIM], BF16, tag="w2")
            nc.gpsimd.dma_start(w2_sb[:], w2aug[:])
            z_aug = big.tile([INNER + 1, RB], BF16, tag="zaug")
            nc.gpsimd.dma_start(z_aug[INNER:INNER + 1, :], onesrow[:])

            # pre-load the Exp activation table off the critical path
            etab = tmp.tile([1, 4], F32, tag="etab", name="etab")
            nc.scalar.activation(etab[:], w1b_sb[0:1, 0:4],
                                 mybir.ActivationFunctionType.Exp)

            def project_half(w_t, proj, rt, bounce=False):
                """proj[:, :, 512rt:512rt+512] = fp8(rope(W @ h^T)).
                DoubleRow fp8 matmuls -> psum f32 -> bf16 staging (scalar)
                -> rope on vector+gpsimd -> fp8 slots (dt, dt+4); do-order
                interleaves the (dt, dt+4) halves so RoPE pairs complete
                (and optionally bounce to DRAM) right behind PE."""
                cols = slice(512 * rt, 512 * (rt + 1))
                pbf = stg.tile([128, NDT, 512], BF16, tag="pbf",
                               name=f"pbf{rt}")

                def rope_pair(dt):
                    # both batches share the same strided rows, so one
                    # cos/sin tile serves rt=0 and rt=1
                    cm = cos_t[dt][:, :]
                    sm = sin_t[dt][:, :]
                    lo = pbf[:, dt, :]
                    hi = pbf[:, dt + NDT // 2, :]
                    ta = tmp.tile([128, 512], BF16, tag="ta", name="ta")
                    tb = tmp.tile([128, 512], BF16, tag="tb", name="tb")
                    td = tmp.tile([128, 512], BF16, tag="td", name="td")
                    # sin-muls on gpsimd: vector was the projection-phase
                    # straggler at 6 DVE ops/pair
                    nc.gpsimd.tensor_mul(tb[:], lo, sm)
                    nc.gpsimd.tensor_mul(td[:], hi, sm)
                    nc.vector.tensor_mul(ta[:], lo, cm)
                    nc.vector.tensor_sub(proj[:, dt, cols], ta[:], td[:])
                    nc.vector.tensor_mul(ta[:], hi, cm)
                    nc.vector.tensor_add(proj[:, dt + NDT // 2, cols],
                                         ta[:], tb[:])
                    if bounce:
                        # both rope slots of the pair in one strided dma
                        eng = nc.sync if dt % 2 else nc.scalar
                        eng.dma_start(
                            kb_bounce[rt][:, dt::NDT // 2, :],
                            proj[:, dt::NDT // 2, cols])

                order = [x for pair in zip(range(NDT // 2),
                                           range(NDT // 2, NDT))
                         for x in pair]            # 0,4,1,5,2,6,3,7
                for do in order:
                    ps = pps.tile([128, 512], F32, tag="ps",
                                  name=f"psp{rt}{do}")
                    for dp in range(NDP):
                        nc.tensor.matmul(
                            ps[:], w_t[dp][:, :, 128 * do:128 * (do + 1)],
                            h_t[dp][:, :, cols],
                            start=(dp == 0), stop=(dp == NDP - 1),
                            perf_mode=DR,
                        )
                    # f32 psum -> bf16 staging for rope (reference casts
                    # q/k to bf16 here); scalar ACT keeps vector free for
                    # rope and unblocks psum banks for the next matmuls
                    nc.scalar.activation(pbf[:, do, :], ps[:],
                                         mybir.ActivationFunctionType.Copy)
                    if do >= NDT // 2:
                        rope_pair(do - NDT // 2)

            # ---- gathered-K load helper ------------------------------------
            g_r = [G[b].rearrange("(r p) t (kb jj) -> r p t kb jj",
                                  r=NCORES, p=128, kb=NSUB, jj=128)
                   for b in range(B)]
            # batch-1 loads stay off the scalar queue: scores(0)'s exps
            # would otherwise be head-of-line blocked behind AG1-gated dmas
            _kteng = {0: [nc.sync, nc.scalar, nc.gpsimd],
                      1: [nc.sync, nc.gpsimd]}

            def load_kt(b, hh):
                # kt layout: [128 (d in tile), t4, kb4, r4, jj128] with a
                # SEPARATE tile per t-half, so score matmuls on dp 0-1
                # start as soon as the first half of the transfer lands;
                # the DoubleRow moving slice [:, 2dp':2dp'+2, kb, :, :]
                # flattens to [128, 2, 512] (kb-major puts the block's
                # (r, jj) columns contiguous in SBUF).
                engs = _kteng[b]
                kts, i = [], hh
                for dh in range(2):
                    kt = big.tile([128, NDT // 2, NSUB, 4, 128], F8,
                                  tag=f"kt{b}{hh}{dh}", name=f"kt{b}{hh}{dh}")
                    for r in range(4):
                        eng = engs[i % len(engs)]
                        i += 1
                        eng.dma_start(
                            kt[:, :, :, r, :],
                            g_r[b][4 * hh + r, :, 4 * dh:4 * (dh + 1), :, :])
                    kts.append(kt)
                return kts

            # ---- k per batch: project+rope+bounce, then both all-gathers
            # back to back on the cc stream; q projects during the
            # collectives; kt loads are emitted last so no engine stream
            # has compute queued behind a gather-gated dma issue ---------
            k_rope = big.tile([128, NDT, RLOC], F8, tag="krope")
            project_half(wk_t, k_rope, 0, bounce=True)
            nc.gpsimd.collective_compute(
                "AllGather", mybir.AluOpType.bypass, replica_groups=groups,
                ins=[kb_bounce[0].ap().opt()], outs=[G[0].ap().opt()])
            project_half(wk_t, k_rope, 1, bounce=True)
            nc.gpsimd.collective_compute(
                "AllGather", mybir.AluOpType.bypass, replica_groups=groups,
                ins=[kb_bounce[1].ap().opt()], outs=[G[1].ap().opt()])

            # ---- q: project + rope (overlaps with the collectives) ----
            q_rope = big.tile([128, NDT, RLOC], F8, tag="qrope")
            project_half(wq_t, q_rope, 0)
            project_half(wq_t, q_rope, 1)

            # ---- PE warmers: vector-paced dummy matmuls keep the HAM
            # clock gate open across the AG0 wait (~1.4us period each) ---
            wsb = big.tile([128, 512], BF16, tag="wsb")
            nc.vector.tensor_copy(wsb[:], cos_t[0][:])

            def warmers(n, pfx):
                for i in range(n):
                    po = ppo.tile([128, 512], F32, tag="po",
                                  name=f"warm{pfx}{i}")
                    nc.tensor.matmul(po[:], cos_t[0][:, 0:128], wsb[:],
                                     start=True, stop=True)
                    nc.vector.tensor_copy(wsb[:], po[:])

            warmers(NWARM_GAP, "g")

            kt0 = [load_kt(0, hh) for hh in range(2)]
            kt1 = [load_kt(1, hh) for hh in range(2)]

            o_sb = big.tile([128, B * NSUB], F32, tag="o")

            def scores(b, kts):
                rs_t = [rsp.tile([128, 2 * NSUB], F32, tag=f"rs{s}",
                                 name=f"rs{b}{s}") for s in range(NSUB)]
                for hh in range(2):
                    for s in range(NSUB):
                        blocks = list(range(s + 1))
                        for c0 in range(0, len(blocks), CHUNK):
                            chunk = blocks[c0:c0 + CHUNK]
                            psl = [pps.tile([128, 512], F32, tag="ps",
                                            name=f"ps{b}{hh}{s}{c0}_{i}")
                                   for i in range(len(chunk))]
                            for dp in range(NDP):
                                lhsT = q_rope[:, 2 * dp:2 * dp + 2,
                                              RB * b + 128 * s:
                                              RB * b + 128 * (s + 1)]
                                dpl = 2 * (dp % 2)
                                for kb, ps in zip(chunk, psl):
                                    nc.tensor.matmul(
                                        ps[:], lhsT,
                                        kts[hh][dp // 2][:, dpl:dpl + 2,
                                                         kb, :, :],
                                        start=(dp == 0), stop=(dp == NDP - 1),
                                        perf_mode=DR,
                                    )
                            for kb, ps in zip(chunk, psl):
                                if kb == s:
                                    nc.vector.tensor_add(ps[:], ps[:],
                                                         mask_sb[hh][:])
                                nc.scalar.activation(
                                    ps[:], ps[:],
                                    mybir.ActivationFunctionType.Exp,
                                    scale=SCALE,
                                    accum_out=rs_t[s][:, 2 * kb + hh:
                                                      2 * kb + hh + 1],
                                )
                for s in range(NSUB):
                    nc.vector.reduce_sum(
                        o_sb[:, NSUB * b + s:NSUB * b + s + 1],
                        rs_t[s][:, 0:2 * (s + 1)], axis=mybir.AxisListType.X)
                    mlp_sub(b, s)

            def mlp_sub(b, s):
                # o_sb[p, b*NSUB+s] is local row b*RB + 128s + p.
                # z[row, n] = relu(o[row]*w1[n] + b1[n]) with o as a
                # per-partition scalar, DVE-transposed into z_aug[n, row],
                # then out rows = z_aug.T @ w2aug.
                col = NSUB * b + s
                zrow = tmp.tile([128, 32], F32, tag="zr", name=f"zr{b}{s}")
                nc.vector.tensor_scalar_mul(zrow[:], w1b_sb[:],
                                            o_sb[:, col:col + 1])
                nc.vector.tensor_add(zrow[:], zrow[:], b1b_sb[:])
                zrb = tmp.tile([128, 32], BF16, tag="zrb",
                               name=f"zrb{b}{s}")
                nc.vector.tensor_scalar_max(zrb[:], zrow[:], 0.0)
                zts = tmp.tile([32, 128], BF16, tag="zts", name=f"zts{b}{s}")
                for g in range(4):
                    nc.vector.transpose(zts[0:32, 32 * g:32 * (g + 1)],
                                        zrb[32 * g:32 * (g + 1), :])
                nc.vector.tensor_copy(z_aug[0:INNER, 128 * s:128 * (s + 1)],
                                      zts[0:INNER, :])
                row0 = RB * b + 128 * s
                ob = obp.tile([128, DIM], BF16, tag="ob", name=f"ob{b}{s}")
                for hh in range(2):
                    po = ppo.tile([128, 512], F32, tag="po",
                                  name=f"po{b}{s}{hh}")
                    nc.tensor.matmul(po[:],
                                     z_aug[:, 128 * s:128 * (s + 1)],
                                     w2_sb[:, 512 * hh:512 * (hh + 1)],
                                     start=True, stop=True)
                    nc.vector.tensor_copy(ob[:, 512 * hh:512 * (hh + 1)],
                                          po[:])
                eng = nc.gpsimd if s % 2 else nc.sync
                eng.dma_start(out_d[row0:row0 + 128, :], ob[:])

            scores(0, kt0)
            warmers(NWARM_MID, "m")
            scores(1, kt1)

    nc.compile()
    return nc


def get_nc():
    if "nc" not in _NC_CACHE:
        _NC_CACHE["nc"] = _build_nc()
    return _NC_CACHE["nc"]


def make_in_maps(hidden_states, Wq, Wk, fc1_w, fc1_b, fc2_w, fc2_b):
    hidden_states = np.asarray(hidden_states, dtype=np.float32)
    Wq = np.asarray(Wq, dtype=np.float32)
    Wk = np.asarray(Wk, dtype=np.float32)
    fc1_w = np.asarray(fc1_w, dtype=np.float32)
    fc1_b = np.asarray(fc1_b, dtype=np.float32)
    fc2_w = np.asarray(fc2_w, dtype=np.float32)
    fc2_b = np.asarray(fc2_b, dtype=np.float32)

    wqT = np.ascontiguousarray(Wq.T).astype(float8_e4m3)
    wkT = np.ascontiguousarray(Wk.T).astype(float8_e4m3)
    w1b = np.zeros((128, 32), dtype=np.float32)
    w1b[:, 0:INNER] = fc1_w.reshape(1, INNER)
    b1b = np.zeros((128, 32), dtype=np.float32)
    b1b[:, 0:INNER] = fc1_b.reshape(1, INNER)
    w2aug = np.concatenate([fc2_w.T, fc2_b[None, :]], axis=0).astype(bfloat16)

    inv_freq = ROPE_BASE ** (-np.arange(0, DIM, 2, dtype=np.float32) / DIM)

    in_maps = []
    for c in range(NCORES):
        rows = np.arange(RB) * NCORES + c            # global rows, per batch
        hT = np.concatenate(
            [hidden_states[b, rows, :].T for b in range(B)],
            axis=1).astype(float8_e4m3)              # [DIM, RLOC]
        ang = rows[:, None].astype(np.float32) * inv_freq[None, :]  # [RB,512]
        cosh = np.ascontiguousarray(np.cos(ang).T).astype(bfloat16)
        sinh = np.ascontiguousarray(np.sin(ang).T).astype(bfloat16)
        # mask_h[p, (jc-4h)*128+t]: allow k col (rank jc, t) for q row p iff
        # 8t + jc <= 8p + c  (boundary subtile; same for every s and batch)
        p = np.arange(128)[:, None, None]
        t = np.arange(128)[None, None, :]
        masks = []
        for h in range(2):
            jc = (np.arange(4) + 4 * h)[None, :, None]
            allow = (NCORES * t + jc) <= (NCORES * p + c)
            masks.append(np.where(allow, 0.0, MASK_NEG)
                         .astype(np.float32).reshape(128, 512))
        in_maps.append({
            "hT": np.ascontiguousarray(hT),
            "wqT": wqT, "wkT": wkT,
            "cosh": cosh, "sinh": sinh,
            "mask0": masks[0], "mask1": masks[1],
            "w1b": w1b, "b1b": b1b, "w2aug": w2aug,
            "onesrow": np.ones((1, RB), dtype=bfloat16),
        })
    return in_maps


def assemble_output(results):
    out = np.empty((B, L, DIM), dtype=np.float32)
    for c in range(NCORES):
        for b in range(B):
            out[b, c::NCORES, :] = (
                results[c]["out"][RB * b:RB * (b + 1)].astype(np.float32))
    return out


def run(trace=False, **inputs):
    nc = get_nc()
    in_maps = make_in_maps(**inputs)
    res = run_bass_kernel_spmd(nc, in_maps, core_ids=list(range(NCORES)),
                               trace=trace)
    return assemble_output(res.results), res


def kernel(**inputs) -> np.ndarray:
    out, _ = run(trace=False, **inputs)
    return out


# revision 12
# speedup vs baseline: 1.1259x; 1.0199x over previous
"""Distributed Trainium2 kernel for nn_AddAttention_154618823089.

Computation (see reference):
    q = rope(bf16(hidden @ Wq.T)); k = rope(bf16(hidden @ Wk.T))
    o[b,l] = sum_{j<=l} exp(q_l . k_j / sqrt(DIM))          (no softmax norm)
    out = relu(o @ fc1_w.T + fc1_b) @ fc2_w.T + fc2_b

Sharding: every core c handles the strided row set {r : r % 8 == c} of
BOTH batches (512 rows each).  Striding makes the causal workload identical
on every core, and taking rows from both batches makes the k exchange a
single fast 8-rank RDH AllGather per batch (4-rank groups fall back to the
slow Mesh path; finer splits pay a ~20us per-op cc cost).

v4 (over the 202us baseline):
  - fp8 DoubleRow pipeline as before (projections, score matmuls); exp
    fused with row-sum via accum_out; MLP per subtile
  - RoPE sin-muls moved to gpsimd so the vector queue stops lagging the
    PE by ~16us at the end of the projection phase
  - PE warmer chain (vector-paced dummy matmuls) spans the AG0 wait and
    the scores(0)->scores(1) gap so the HAM clock gate keeps the PE at
    2.4GHz instead of 1.2GHz through the scores phases
  - bf16 output (cast to f32 on host): halves output HBM traffic that
    competes with AllGather1
  - cos/sin loads halved (both batches share the same strided rows)
  - Exp activation table preloaded off the critical path
  - batch-1 kt loads stay OFF the scalar queue (scores(0) exps would be
    head-of-line blocked behind their AG1-gated DMAs)
"""

import sys
import types

import numpy as np
from ml_dtypes import bfloat16, float8_e4m3

import concourse.bacc as bacc
import concourse.bass as bass
import concourse.mybir as mybir
import concourse.tile as tile
from concourse.bass_utils import run_bass_kernel_spmd


def _install_ntff_hook():
    """The container's antenv lacks axon_hooks; provide it so trace=True can
    capture NTFF profiles (exec_time_ns) through the axon PJRT library."""
    if "antenv.axon_hooks" in sys.modules:
        return
    try:
        sys.path.insert(0, "/root/.axon_site/trn_agent_boot")
        import trn_boot

        mod = types.ModuleType("antenv.axon_hooks")
        _h = {"hook": None}
        mod.set_axon_ntff_profile_hook = lambda h: _h.__setitem__("hook", h)
        mod.get_axon_ntff_profile_hook = lambda: _h["hook"]
        sys.modules["antenv.axon_hooks"] = mod
        import antenv

        antenv.axon_hooks = mod
        mod.set_axon_ntff_profile_hook(
            trn_boot._ntff_profile_via_ctypes("/opt/axon/libaxon_pjrt.so"))
    except Exception:
        pass


_install_ntff_hook()

B, L, DIM, INNER = 2, 4096, 1024, 16
ROPE_BASE = 32.0
NCORES = 8
RB = L // NCORES       # rows per core per batch (512)
RLOC = 2 * RB          # local q/k rows per core (both batches, 1024)
NSUB = RB // 128       # q subtiles per core per batch (4)
NDT = DIM // 128       # d tiles (8)
NDP = NDT // 2         # DoubleRow d-tile pairs (4)
SCALE = 1.0 / float(np.sqrt(DIM))
MASK_NEG = -1.0e6
CHUNK = 3              # psum banks per score chunk
NWARM_GAP = 24         # PE warmers across the AG0 wait
NWARM_MID = 9          # PE warmers across the scores(0)->scores(1) gap
F32 = mybir.dt.float32
BF16 = mybir.dt.bfloat16
F8 = mybir.dt.float8e4
DR = mybir.MatmulPerfMode.DoubleRow

_NC_CACHE = {}


def _build_nc():
    nc = bacc.Bacc("TRN2", target_bir_lowering=False, debug=False,
                   num_devices=NCORES, num_swdge_queues=4)

    hT = nc.dram_tensor("hT", [DIM, RLOC], F8, kind="ExternalInput")
    wqT = nc.dram_tensor("wqT", [DIM, DIM], F8, kind="ExternalInput")
    wkT = nc.dram_tensor("wkT", [DIM, DIM], F8, kind="ExternalInput")
    cosh = nc.dram_tensor("cosh", [DIM // 2, RB], BF16, kind="ExternalInput")
    sinh = nc.dram_tensor("sinh", [DIM // 2, RB], BF16, kind="ExternalInput")
    mask0 = nc.dram_tensor("mask0", [128, 512], F32, kind="ExternalInput")
    mask1 = nc.dram_tensor("mask1", [128, 512], F32, kind="ExternalInput")
    w1b_d = nc.dram_tensor("w1b", [128, 32], F32, kind="ExternalInput")
    b1b_d = nc.dram_tensor("b1b", [128, 32], F32, kind="ExternalInput")
    w2aug = nc.dram_tensor("w2aug", [INNER + 1, DIM], BF16, kind="ExternalInput")
    onesrow = nc.dram_tensor("onesrow", [1, RB], BF16, kind="ExternalInput")
    out_d = nc.dram_tensor("out", [RLOC, DIM], BF16, kind="ExternalOutput")

    # one bounce + AllGather per batch: small collectives pay a ~20us
    # fixed per-op cost on the cc stream, so two 4MB-out gathers beat any
    # finer split
    kb_bounce = [nc.dram_tensor(f"kTb{b}", [128, NDT, RB], F8)
                 for b in range(B)]
    G = [nc.dram_tensor(f"G{b}", [NCORES * 128, NDT, RB], F8,
                        addr_space="Shared") for b in range(B)]

    groups = [list(range(NCORES))]

    with tile.TileContext(nc) as tc:
        with (
            tc.tile_pool(name="big", bufs=1) as big,
            tc.tile_pool(name="tmp", bufs=2) as tmp,
            tc.tile_pool(name="stg", bufs=2) as stg,
            tc.tile_pool(name="rsp", bufs=2) as rsp,
            tc.tile_pool(name="obp", bufs=4) as obp,
            tc.tile_pool(name="ps", bufs=7, space="PSUM") as pps,
            tc.tile_pool(name="po", bufs=1, space="PSUM") as ppo,
        ):
            # ---- inputs -> SBUF as DoubleRow pair tiles, spread on queues --
            # (dp p k2 r) views land each pair tile in ONE dma each
            h_r = hT.rearrange("(dp k2 p) r -> dp p k2 r", dp=NDP, k2=2, p=128)
            wk_r = wkT.rearrange("(dp k2 p) r -> dp p k2 r",
                                 dp=NDP, k2=2, p=128)
            wq_r = wqT.rearrange("(dp k2 p) r -> dp p k2 r",
                                 dp=NDP, k2=2, p=128)
            h_t, wk_t, wq_t = [], [], []
            for dp in range(NDP):
                th = big.tile([128, 2, RLOC], F8, tag=f"h{dp}", name=f"h{dp}")
                nc.sync.dma_start(th[:], h_r[dp])
                h_t.append(th)
                tw = big.tile([128, 2, DIM], F8, tag=f"wk{dp}", name=f"wk{dp}")
                nc.scalar.dma_start(tw[:], wk_r[dp])
                wk_t.append(tw)
            cos_t, sin_t = [], []
            for ci in range(NDT // 2):
                tc_ = big.tile([128, RB], BF16, tag=f"cos{ci}",
                               name=f"cos{ci}")
                nc.sync.dma_start(tc_[:], cosh[128 * ci:128 * (ci + 1), :])
                cos_t.append(tc_)
                ts_ = big.tile([128, RB], BF16, tag=f"sin{ci}",
                               name=f"sin{ci}")
                nc.scalar.dma_start(ts_[:], sinh[128 * ci:128 * (ci + 1), :])
                sin_t.append(ts_)
            for dp in range(NDP):
                # wq reuses wk's slots (k projection is done by then)
                tw = big.tile([128, 2, DIM], F8, tag=f"wk{dp}", name=f"wq{dp}")
                nc.sync.dma_start(tw[:], wq_r[dp])
                wq_t.append(tw)
            mask_sb = [big.tile([128, 512], F32, tag=f"mask{h}",
                                name=f"mask_sb{h}") for h in range(2)]
            nc.gpsimd.dma_start(mask_sb[0][:], mask0[:])
            nc.gpsimd.dma_start(mask_sb[1][:], mask1[:])
            w1b_sb = big.tile([128, 32], F32, tag="w1b")
            nc.gpsimd.dma_start(w1b_sb[:], w1b_d[:])
            b1b_sb = big.tile([128, 32], F32, tag="b1b")
            nc.gpsimd.dma_start(b1b_sb[:], b1b_d[:])
            w2_sb = big.tile([INNER + 1, DIM], BF16, tag="w2")
            nc.gpsimd.dma_start(w2_sb[:], w2aug[:])
            z_aug = big.tile([INNER + 1, RB], BF16, tag="zaug")
            nc.gpsimd.dma_start(z_aug[INNER:INNER + 1, :], onesrow[:])

            # pre-load the Exp activation table off the critical path
            etab = tmp.tile([1, 4], F32, tag="etab", name="etab")
            nc.scalar.activation(etab[:], w1b_sb[0:1, 0:4],
                                 mybir.ActivationFunctionType.Exp)

            def project_half(w_t, proj, rt, bounce=False):
                """proj[:, :, 512rt:512rt+512] = fp8(rope(W @ h^T)).
                DoubleRow fp8 matmuls -> psum f32 -> bf16 staging (scalar)
                -> rope on vector+gpsimd -> fp8 slots (dt, dt+4); do-order
                interleaves the (dt, dt+4) halves so RoPE pairs complete
                (and optionally bounce to DRAM) right behind PE."""
                cols = slice(512 * rt, 512 * (rt + 1))
                pbf = stg.tile([128, NDT, 512], BF16, tag="pbf",
                               name=f"pbf{rt}")

                def rope_pair(dt):
                    # both batches share the same strided rows, so one
                    # cos/sin tile serves rt=0 and rt=1
                    cm = cos_t[dt][:, :]
                    sm = sin_t[dt][:, :]
                    lo = pbf[:, dt, :]
                    hi = pbf[:, dt + NDT // 2, :]
                    ta = tmp.tile([128, 512], BF16, tag="ta", name="ta")
                    tb = tmp.tile([128, 512], BF16, tag="tb", name="tb")
                    td = tmp.tile([128, 512], BF16, tag="td", name="td")
                    # all on vector: gpsimd elementwise is 3x slower AND
                    # anything queued on gpsimd behind a collective trigger
                    # blocks until the cc stream accepts that trigger
                    nc.vector.tensor_mul(ta[:], lo, cm)
                    nc.vector.tensor_mul(tb[:], lo, sm)
                    nc.vector.tensor_mul(td[:], hi, sm)
                    nc.vector.tensor_sub(proj[:, dt, cols], ta[:], td[:])
                    nc.vector.tensor_mul(ta[:], hi, cm)
                    nc.vector.tensor_add(proj[:, dt + NDT // 2, cols],
                                         ta[:], tb[:])
                    if bounce:
                        # both rope slots of the pair in one strided dma
                        eng = nc.sync if dt % 2 else nc.scalar
                        eng.dma_start(
                            kb_bounce[rt][:, dt::NDT // 2, :],
                            proj[:, dt::NDT // 2, cols])

                order = [x for pair in zip(range(NDT // 2),
                                           range(NDT // 2, NDT))
                         for x in pair]            # 0,4,1,5,2,6,3,7
                for do in order:
                    ps = pps.tile([128, 512], F32, tag="ps",
                                  name=f"psp{rt}{do}")
                    for dp in range(NDP):
                        nc.tensor.matmul(
                            ps[:], w_t[dp][:, :, 128 * do:128 * (do + 1)],
                            h_t[dp][:, :, cols],
                            start=(dp == 0), stop=(dp == NDP - 1),
                            perf_mode=DR,
                        )
                    # f32 psum -> bf16 staging for rope (reference casts
                    # q/k to bf16 here); scalar ACT keeps vector free for
                    # rope and unblocks psum banks for the next matmuls
                    nc.scalar.activation(pbf[:, do, :], ps[:],
                                         mybir.ActivationFunctionType.Copy)
                    if do >= NDT // 2:
                        rope_pair(do - NDT // 2)

            # ---- gathered-K load helper ------------------------------------
            g_r = [G[b].rearrange("(r p) t (kb jj) -> r p t kb jj",
                                  r=NCORES, p=128, kb=NSUB, jj=128)
                   for b in range(B)]
            # batch-1 loads stay off the scalar queue: scores(0)'s exps
            # would otherwise be head-of-line blocked behind AG1-gated dmas
            _kteng = {0: [nc.sync, nc.scalar, nc.gpsimd],
                      1: [nc.sync, nc.gpsimd]}

            def load_kt(b, hh):
                # kt layout: [128 (d in tile), t4, kb4, r4, jj128] with a
                # SEPARATE tile per t-half, so score matmuls on dp 0-1
                # start as soon as the first half of the transfer lands;
                # the DoubleRow moving slice [:, 2dp':2dp'+2, kb, :, :]
                # flattens to [128, 2, 512] (kb-major puts the block's
                # (r, jj) columns contiguous in SBUF).
                engs = _kteng[b]
                kts, i = [], hh
                for dh in range(2):
                    kt = big.tile([128, NDT // 2, NSUB, 4, 128], F8,
                                  tag=f"kt{b}{hh}{dh}", name=f"kt{b}{hh}{dh}")
                    for r in range(4):
                        eng = engs[i % len(engs)]
                        i += 1
                        eng.dma_start(
                            kt[:, :, :, r, :],
                            g_r[b][4 * hh + r, :, 4 * dh:4 * (dh + 1), :, :])
                    kts.append(kt)
                return kts

            # ---- k per batch: project+rope+bounce, then both all-gathers
            # back to back on the cc stream; q projects during the
            # collectives; kt loads are emitted last so no engine stream
            # has compute queued behind a gather-gated dma issue ---------
            k_rope = big.tile([128, NDT, RLOC], F8, tag="krope")
            project_half(wk_t, k_rope, 0, bounce=True)
            nc.gpsimd.collective_compute(
                "AllGather", mybir.AluOpType.bypass, replica_groups=groups,
                ins=[kb_bounce[0].ap().opt()], outs=[G[0].ap().opt()])
            project_half(wk_t, k_rope, 1, bounce=True)
            nc.gpsimd.collective_compute(
                "AllGather", mybir.AluOpType.bypass, replica_groups=groups,
                ins=[kb_bounce[1].ap().opt()], outs=[G[1].ap().opt()])

            # ---- q: project + rope (overlaps with the collectives) ----
            q_rope = big.tile([128, NDT, RLOC], F8, tag="qrope")
            project_half(wq_t, q_rope, 0)
            project_half(wq_t, q_rope, 1)

            # ---- PE warmers: vector-paced dummy matmuls keep the HAM
            # clock gate open across the AG0 wait (~1.4us period each) ---
            wsb = big.tile([128, 512], BF16, tag="wsb")
            nc.vector.tensor_copy(wsb[:], cos_t[0][:])

            def warmers(n, pfx):
                for i in range(n):
                    po = ppo.tile([128, 512], F32, tag="po",
                                  name=f"warm{pfx}{i}")
                    nc.tensor.matmul(po[:], cos_t[0][:, 0:128], wsb[:],
                                     start=True, stop=True)
                    nc.vector.tensor_copy(wsb[:], po[:])

            warmers(NWARM_GAP, "g")

            kt0 = [load_kt(0, hh) for hh in range(2)]
            kt1 = [load_kt(1, hh) for hh in range(2)]

            o_sb = big.tile([128, B * NSUB], F32, tag="o")

            def scores(b, kts):
                rs_t = [rsp.tile([128, 2 * NSUB], F32, tag=f"rs{s}",
                                 name=f"rs{b}{s}") for s in range(NSUB)]
                for hh in range(2):
                    for s in range(NSUB):
                        blocks = list(range(s + 1))
                        for c0 in range(0, len(blocks), CHUNK):
                            chunk = blocks[c0:c0 + CHUNK]
                            psl = [pps.tile([128, 512], F32, tag="ps",
                                            name=f"ps{b}{hh}{s}{c0}_{i}")
                                   for i in range(len(chunk))]
                            for dp in range(NDP):
                                lhsT = q_rope[:, 2 * dp:2 * dp + 2,
                                              RB * b + 128 * s:
                                              RB * b + 128 * (s + 1)]
                                dpl = 2 * (dp % 2)
                                for kb, ps in zip(chunk, psl):
                                    nc.tensor.matmul(
                                        ps[:], lhsT,
                                        kts[hh][dp // 2][:, dpl:dpl + 2,
                                                         kb, :, :],
                                        start=(dp == 0), stop=(dp == NDP - 1),
                                        perf_mode=DR,
                                    )
                            for kb, ps in zip(chunk, psl):
                                if kb == s:
                                    nc.vector.tensor_add(ps[:], ps[:],
                                                         mask_sb[hh][:])
                                nc.scalar.activation(
                                    ps[:], ps[:],
                                    mybir.ActivationFunctionType.Exp,
                                    scale=SCALE,
                                    accum_out=rs_t[s][:, 2 * kb + hh:
                                                      2 * kb + hh + 1],
                                )
                for s in range(NSUB):
                    nc.vector.reduce_sum(
                        o_sb[:, NSUB * b + s:NSUB * b + s + 1],
                        rs_t[s][:, 0:2 * (s + 1)], axis=mybir.AxisListType.X)
                    mlp_sub(b, s)

            def mlp_sub(b, s):
                # o_sb[p, b*NSUB+s] is local row b*RB + 128s + p.
                # z[row, n] = relu(o[row]*w1[n] + b1[n]) with o as a
                # per-partition scalar, DVE-transposed into z_aug[n, row],
                # then out rows = z_aug.T @ w2aug.
                col = NSUB * b + s
                zrow = tmp.tile([128, 32], F32, tag="zr", name=f"zr{b}{s}")
                nc.vector.tensor_scalar_mul(zrow[:], w1b_sb[:],
                                            o_sb[:, col:col + 1])
                nc.vector.tensor_add(zrow[:], zrow[:], b1b_sb[:])
                zrb = tmp.tile([128, 32], BF16, tag="zrb",
                               name=f"zrb{b}{s}")
                nc.vector.tensor_scalar_max(zrb[:], zrow[:], 0.0)
                zts = tmp.tile([32, 128], BF16, tag="zts", name=f"zts{b}{s}")
                for g in range(4):
                    nc.vector.transpose(zts[0:32, 32 * g:32 * (g + 1)],
                                        zrb[32 * g:32 * (g + 1), :])
                nc.vector.tensor_copy(z_aug[0:INNER, 128 * s:128 * (s + 1)],
                                      zts[0:INNER, :])
                row0 = RB * b + 128 * s
                ob = obp.tile([128, DIM], BF16, tag="ob", name=f"ob{b}{s}")
                for hh in range(2):
                    po = ppo.tile([128, 512], F32, tag="po",
                                  name=f"po{b}{s}{hh}")
                    nc.tensor.matmul(po[:],
                                     z_aug[:, 128 * s:128 * (s + 1)],
                                     w2_sb[:, 512 * hh:512 * (hh + 1)],
                                     start=True, stop=True)
                    nc.vector.tensor_copy(ob[:, 512 * hh:512 * (hh + 1)],
                                          po[:])
                eng = nc.gpsimd if s % 2 else nc.sync
                eng.dma_start(out_d[row0:row0 + 128, :], ob[:])

            scores(0, kt0)
            warmers(NWARM_MID, "m")
            scores(1, kt1)

    nc.compile()
    return nc


def get_nc():
    if "nc" not in _NC_CACHE:
        _NC_CACHE["nc"] = _build_nc()
    return _NC_CACHE["nc"]


def make_in_maps(hidden_states, Wq, Wk, fc1_w, fc1_b, fc2_w, fc2_b):
    hidden_states = np.asarray(hidden_states, dtype=np.float32)
    Wq = np.asarray(Wq, dtype=np.float32)
    Wk = np.asarray(Wk, dtype=np.float32)
    fc1_w = np.asarray(fc1_w, dtype=np.float32)
    fc1_b = np.asarray(fc1_b, dtype=np.float32)
    fc2_w = np.asarray(fc2_w, dtype=np.float32)
    fc2_b = np.asarray(fc2_b, dtype=np.float32)

    wqT = np.ascontiguousarray(Wq.T).astype(float8_e4m3)
    wkT = np.ascontiguousarray(Wk.T).astype(float8_e4m3)
    w1b = np.zeros((128, 32), dtype=np.float32)
    w1b[:, 0:INNER] = fc1_w.reshape(1, INNER)
    b1b = np.zeros((128, 32), dtype=np.float32)
    b1b[:, 0:INNER] = fc1_b.reshape(1, INNER)
    w2aug = np.concatenate([fc2_w.T, fc2_b[None, :]], axis=0).astype(bfloat16)

    inv_freq = ROPE_BASE ** (-np.arange(0, DIM, 2, dtype=np.float32) / DIM)

    in_maps = []
    for c in range(NCORES):
        rows = np.arange(RB) * NCORES + c            # global rows, per batch
        hT = np.concatenate(
            [hidden_states[b, rows, :].T for b in range(B)],
            axis=1).astype(float8_e4m3)              # [DIM, RLOC]
        ang = rows[:, None].astype(np.float32) * inv_freq[None, :]  # [RB,512]
        cosh = np.ascontiguousarray(np.cos(ang).T).astype(bfloat16)
        sinh = np.ascontiguousarray(np.sin(ang).T).astype(bfloat16)
        # mask_h[p, (jc-4h)*128+t]: allow k col (rank jc, t) for q row p iff
        # 8t + jc <= 8p + c  (boundary subtile; same for every s and batch)
        p = np.arange(128)[:, None, None]
        t = np.arange(128)[None, None, :]
        masks = []
        for h in range(2):
            jc = (np.arange(4) + 4 * h)[None, :, None]
            allow = (NCORES * t + jc) <= (NCORES * p + c)
            masks.append(np.where(allow, 0.0, MASK_NEG)
                         .astype(np.float32).reshape(128, 512))
        in_maps.append({
            "hT": np.ascontiguousarray(hT),
            "wqT": wqT, "wkT": wkT,
            "cosh": cosh, "sinh": sinh,
            "mask0": masks[0], "mask1": masks[1],
            "w1b": w1b, "b1b": b1b, "w2aug": w2aug,
            "onesrow": np.ones((1, RB), dtype=bfloat16),
        })
    return in_maps


def assemble_output(results):
    out = np.empty((B, L, DIM), dtype=np.float32)
    for c in range(NCORES):
        for b in range(B):
            out[b, c::NCORES, :] = (
                results[c]["out"][RB * b:RB * (b + 1)].astype(np.float32))
    return out


def run(trace=False, **inputs):
    nc = get_nc()
    in_maps = make_in_maps(**inputs)
    res = run_bass_kernel_spmd(nc, in_maps, core_ids=list(range(NCORES)),
                               trace=trace)
    return assemble_output(res.results), res


def kernel(**inputs) -> np.ndarray:
    out, _ = run(trace=False, **inputs)
    return out


# revision 14
# speedup vs baseline: 1.1638x; 1.0337x over previous
"""Distributed Trainium2 kernel for nn_AddAttention_154618823089.

Computation (see reference):
    q = rope(bf16(hidden @ Wq.T)); k = rope(bf16(hidden @ Wk.T))
    o[b,l] = sum_{j<=l} exp(q_l . k_j / sqrt(DIM))          (no softmax norm)
    out = relu(o @ fc1_w.T + fc1_b) @ fc2_w.T + fc2_b

Sharding: every core c handles the strided row set {r : r % 8 == c} of
BOTH batches (512 rows each).  Striding makes the causal workload identical
on every core, and taking rows from both batches makes the k exchange a
single fast 8-rank RDH AllGather per batch (4-rank groups fall back to the
slow Mesh path; finer splits pay a ~20us per-op cc cost).

v4 (over the 202us baseline):
  - fp8 DoubleRow pipeline as before (projections, score matmuls); exp
    fused with row-sum via accum_out; MLP per subtile
  - RoPE sin-muls moved to gpsimd so the vector queue stops lagging the
    PE by ~16us at the end of the projection phase
  - PE warmer chain (vector-paced dummy matmuls) spans the AG0 wait and
    the scores(0)->scores(1) gap so the HAM clock gate keeps the PE at
    2.4GHz instead of 1.2GHz through the scores phases
  - bf16 output (cast to f32 on host): halves output HBM traffic that
    competes with AllGather1
  - cos/sin loads halved (both batches share the same strided rows)
  - Exp activation table preloaded off the critical path
  - batch-1 kt loads stay OFF the scalar queue (scores(0) exps would be
    head-of-line blocked behind their AG1-gated DMAs)
"""

import sys
import types

import numpy as np
from ml_dtypes import bfloat16, float8_e4m3

import concourse.bacc as bacc
import concourse.bass as bass
import concourse.mybir as mybir
import concourse.tile as tile
from concourse.bass_utils import run_bass_kernel_spmd


def _install_ntff_hook():
    """The container's antenv lacks axon_hooks; provide it so trace=True can
    capture NTFF profiles (exec_time_ns) through the axon PJRT library."""
    if "antenv.axon_hooks" in sys.modules:
        return
    try:
        sys.path.insert(0, "/root/.axon_site/trn_agent_boot")
        import trn_boot

        mod = types.ModuleType("antenv.axon_hooks")
        _h = {"hook": None}
        mod.set_axon_ntff_profile_hook = lambda h: _h.__setitem__("hook", h)
        mod.get_axon_ntff_profile_hook = lambda: _h["hook"]
        sys.modules["antenv.axon_hooks"] = mod
        import antenv

        antenv.axon_hooks = mod
        mod.set_axon_ntff_profile_hook(
            trn_boot._ntff_profile_via_ctypes("/opt/axon/libaxon_pjrt.so"))
    except Exception:
        pass


_install_ntff_hook()

B, L, DIM, INNER = 2, 4096, 1024, 16
ROPE_BASE = 32.0
NCORES = 8
RB = L // NCORES       # rows per core per batch (512)
RLOC = 2 * RB          # local q/k rows per core (both batches, 1024)
NSUB = RB // 128       # q subtiles per core per batch (4)
NDT = DIM // 128       # d tiles (8)
NDP = NDT // 2         # DoubleRow d-tile pairs (4)
SCALE = 1.0 / float(np.sqrt(DIM))
MASK_NEG = -1.0e6
CHUNK = 3              # psum banks per score chunk
NWARM_GAP = 200        # self-paced PE warmer matmuls across the AG0 wait
NWARM_MID = 70         # same, across the scores(0)->scores(1) gap
F32 = mybir.dt.float32
BF16 = mybir.dt.bfloat16
F8 = mybir.dt.float8e4
DR = mybir.MatmulPerfMode.DoubleRow

_NC_CACHE = {}


def _build_nc():
    nc = bacc.Bacc("TRN2", target_bir_lowering=False, debug=False,
                   num_devices=NCORES, num_swdge_queues=4)

    hT = nc.dram_tensor("hT", [DIM, RLOC], F8, kind="ExternalInput")
    wqT = nc.dram_tensor("wqT", [DIM, DIM], F8, kind="ExternalInput")
    wkT = nc.dram_tensor("wkT", [DIM, DIM], F8, kind="ExternalInput")
    cosh = nc.dram_tensor("cosh", [DIM // 2, RB], BF16, kind="ExternalInput")
    sinh = nc.dram_tensor("sinh", [DIM // 2, RB], BF16, kind="ExternalInput")
    mask0 = nc.dram_tensor("mask0", [128, 512], F32, kind="ExternalInput")
    mask1 = nc.dram_tensor("mask1", [128, 512], F32, kind="ExternalInput")
    w1b_d = nc.dram_tensor("w1b", [128, 32], F32, kind="ExternalInput")
    b1b_d = nc.dram_tensor("b1b", [128, 32], F32, kind="ExternalInput")
    w2aug = nc.dram_tensor("w2aug", [INNER + 1, DIM], BF16, kind="ExternalInput")
    onesrow = nc.dram_tensor("onesrow", [1, RB], BF16, kind="ExternalInput")
    out_d = nc.dram_tensor("out", [RLOC, DIM], BF16, kind="ExternalOutput")

    # one bounce + AllGather per batch: small collectives pay a ~20us
    # fixed per-op cost on the cc stream, so two 4MB-out gathers beat any
    # finer split
    kb_bounce = [nc.dram_tensor(f"kTb{b}", [128, NDT, RB], F8)
                 for b in range(B)]
    G = [nc.dram_tensor(f"G{b}", [NCORES * 128, NDT, RB], F8,
                        addr_space="Shared") for b in range(B)]

    groups = [list(range(NCORES))]

    with tile.TileContext(nc) as tc:
        with (
            tc.tile_pool(name="big", bufs=1) as big,
            tc.tile_pool(name="tmp", bufs=2) as tmp,
            tc.tile_pool(name="stg", bufs=2) as stg,
            tc.tile_pool(name="rsp", bufs=2) as rsp,
            tc.tile_pool(name="obp", bufs=4) as obp,
            tc.tile_pool(name="ps", bufs=7, space="PSUM") as pps,
            tc.tile_pool(name="po", bufs=1, space="PSUM") as ppo,
        ):
            # ---- inputs -> SBUF as DoubleRow pair tiles, spread on queues --
            # (dp p k2 r) views land each pair tile in ONE dma each
            h_r = hT.rearrange("(dp k2 p) r -> dp p k2 r", dp=NDP, k2=2, p=128)
            wk_r = wkT.rearrange("(dp k2 p) r -> dp p k2 r",
                                 dp=NDP, k2=2, p=128)
            wq_r = wqT.rearrange("(dp k2 p) r -> dp p k2 r",
                                 dp=NDP, k2=2, p=128)
            h_t, wk_t, wq_t = [], [], []
            for dp in range(NDP):
                th = big.tile([128, 2, RLOC], F8, tag=f"h{dp}", name=f"h{dp}")
                nc.sync.dma_start(th[:], h_r[dp])
                h_t.append(th)
                tw = big.tile([128, 2, DIM], F8, tag=f"wk{dp}", name=f"wk{dp}")
                nc.scalar.dma_start(tw[:], wk_r[dp])
                wk_t.append(tw)
            cos_t, sin_t = [], []
            for ci in range(NDT // 2):
                tc_ = big.tile([128, RB], BF16, tag=f"cos{ci}",
                               name=f"cos{ci}")
                nc.sync.dma_start(tc_[:], cosh[128 * ci:128 * (ci + 1), :])
                cos_t.append(tc_)
                ts_ = big.tile([128, RB], BF16, tag=f"sin{ci}",
                               name=f"sin{ci}")
                nc.scalar.dma_start(ts_[:], sinh[128 * ci:128 * (ci + 1), :])
                sin_t.append(ts_)
            for dp in range(NDP):
                # wq reuses wk's slots (k projection is done by then)
                tw = big.tile([128, 2, DIM], F8, tag=f"wk{dp}", name=f"wq{dp}")
                nc.sync.dma_start(tw[:], wq_r[dp])
                wq_t.append(tw)
            mask_sb = [big.tile([128, 512], F32, tag=f"mask{h}",
                                name=f"mask_sb{h}") for h in range(2)]
            nc.gpsimd.dma_start(mask_sb[0][:], mask0[:])
            nc.gpsimd.dma_start(mask_sb[1][:], mask1[:])
            w1b_sb = big.tile([128, 32], F32, tag="w1b")
            nc.gpsimd.dma_start(w1b_sb[:], w1b_d[:])
            b1b_sb = big.tile([128, 32], F32, tag="b1b")
            nc.gpsimd.dma_start(b1b_sb[:], b1b_d[:])
            w2_sb = big.tile([INNER + 1, DIM], BF16, tag="w2")
            nc.gpsimd.dma_start(w2_sb[:], w2aug[:])
            z_aug = big.tile([INNER + 1, RB], BF16, tag="zaug")
            nc.gpsimd.dma_start(z_aug[INNER:INNER + 1, :], onesrow[:])

            # pre-load the Exp activation table off the critical path
            etab = tmp.tile([1, 4], F32, tag="etab", name="etab")
            nc.scalar.activation(etab[:], w1b_sb[0:1, 0:4],
                                 mybir.ActivationFunctionType.Exp)

            def project_half(w_t, proj, rt, bounce=False):
                """proj[:, :, 512rt:512rt+512] = fp8(rope(W @ h^T)).
                DoubleRow fp8 matmuls -> psum f32 -> bf16 staging (scalar)
                -> rope on vector+gpsimd -> fp8 slots (dt, dt+4); do-order
                interleaves the (dt, dt+4) halves so RoPE pairs complete
                (and optionally bounce to DRAM) right behind PE."""
                cols = slice(512 * rt, 512 * (rt + 1))
                pbf = stg.tile([128, NDT, 512], BF16, tag="pbf",
                               name=f"pbf{rt}")

                def rope_pair(dt):
                    # both batches share the same strided rows, so one
                    # cos/sin tile serves rt=0 and rt=1
                    cm = cos_t[dt][:, :]
                    sm = sin_t[dt][:, :]
                    lo = pbf[:, dt, :]
                    hi = pbf[:, dt + NDT // 2, :]
                    ta = tmp.tile([128, 512], BF16, tag="ta", name="ta")
                    tb = tmp.tile([128, 512], BF16, tag="tb", name="tb")
                    td = tmp.tile([128, 512], BF16, tag="td", name="td")
                    # all on vector: gpsimd elementwise is 3x slower AND
                    # anything queued on gpsimd behind a collective trigger
                    # blocks until the cc stream accepts that trigger
                    nc.vector.tensor_mul(ta[:], lo, cm)
                    nc.vector.tensor_mul(tb[:], lo, sm)
                    nc.vector.tensor_mul(td[:], hi, sm)
                    nc.vector.tensor_sub(proj[:, dt, cols], ta[:], td[:])
                    nc.vector.tensor_mul(ta[:], hi, cm)
                    nc.vector.tensor_add(proj[:, dt + NDT // 2, cols],
                                         ta[:], tb[:])
                    if bounce:
                        # both rope slots of the pair in one strided dma
                        eng = nc.sync if dt % 2 else nc.scalar
                        eng.dma_start(
                            kb_bounce[rt][:, dt::NDT // 2, :],
                            proj[:, dt::NDT // 2, cols])

                order = [x for pair in zip(range(NDT // 2),
                                           range(NDT // 2, NDT))
                         for x in pair]            # 0,4,1,5,2,6,3,7
                for do in order:
                    ps = pps.tile([128, 512], F32, tag="ps",
                                  name=f"psp{rt}{do}")
                    for dp in range(NDP):
                        nc.tensor.matmul(
                            ps[:], w_t[dp][:, :, 128 * do:128 * (do + 1)],
                            h_t[dp][:, :, cols],
                            start=(dp == 0), stop=(dp == NDP - 1),
                            perf_mode=DR,
                        )
                    # f32 psum -> bf16 staging for rope (reference casts
                    # q/k to bf16 here); scalar ACT keeps vector free for
                    # rope and unblocks psum banks for the next matmuls
                    nc.scalar.activation(pbf[:, do, :], ps[:],
                                         mybir.ActivationFunctionType.Copy)
                    if do >= NDT // 2:
                        rope_pair(do - NDT // 2)

            # ---- gathered-K load helper ------------------------------------
            g_r = [G[b].rearrange("(r p) t (kb jj) -> r p t kb jj",
                                  r=NCORES, p=128, kb=NSUB, jj=128)
                   for b in range(B)]
            # batch-1 loads stay off the scalar queue: scores(0)'s exps
            # would otherwise be head-of-line blocked behind AG1-gated dmas
            _kteng = {0: [nc.sync, nc.scalar, nc.gpsimd],
                      1: [nc.sync, nc.gpsimd]}

            def load_kt(b, hh):
                # kt layout: [128 (d in tile), t4, kb4, r4, jj128] with a
                # SEPARATE tile per t-half, so score matmuls on dp 0-1
                # start as soon as the first half of the transfer lands;
                # the DoubleRow moving slice [:, 2dp':2dp'+2, kb, :, :]
                # flattens to [128, 2, 512] (kb-major puts the block's
                # (r, jj) columns contiguous in SBUF).
                engs = _kteng[b]
                kts, i = [], hh
                for dh in range(2):
                    kt = big.tile([128, NDT // 2, NSUB, 4, 128], F8,
                                  tag=f"kt{b}{hh}{dh}", name=f"kt{b}{hh}{dh}")
                    for r in range(4):
                        eng = engs[i % len(engs)]
                        i += 1
                        eng.dma_start(
                            kt[:, :, :, r, :],
                            g_r[b][4 * hh + r, :, 4 * dh:4 * (dh + 1), :, :])
                    kts.append(kt)
                return kts

            # ---- k per batch: project+rope+bounce, then both all-gathers
            # back to back on the cc stream; q projects during the
            # collectives; kt loads are emitted last so no engine stream
            # has compute queued behind a gather-gated dma issue ---------
            k_rope = big.tile([128, NDT, RLOC], F8, tag="krope")
            project_half(wk_t, k_rope, 0, bounce=True)
            nc.gpsimd.collective_compute(
                "AllGather", mybir.AluOpType.bypass, replica_groups=groups,
                ins=[kb_bounce[0].ap().opt()], outs=[G[0].ap().opt()])
            project_half(wk_t, k_rope, 1, bounce=True)
            nc.gpsimd.collective_compute(
                "AllGather", mybir.AluOpType.bypass, replica_groups=groups,
                ins=[kb_bounce[1].ap().opt()], outs=[G[1].ap().opt()])

            # ---- q: project + rope (overlaps with the collectives) ----
            q_rope = big.tile([128, NDT, RLOC], F8, tag="qrope")
            project_half(wq_t, q_rope, 0)
            project_half(wq_t, q_rope, 1)

            # ---- PE warmers: a self-paced chain of bf16 matmuls
            # accumulating into one psum bank (~0.21us each back-to-back,
            # no cross-engine deps) keeps the HAM clock gate open across
            # the AG0 wait so scores start at 2.4GHz, not 1.2 -----------
            def warmers(n, pfx):
                po = ppo.tile([128, 512], F32, tag="po", name=f"warm{pfx}")
                for i in range(n):
                    nc.tensor.matmul(po[:], cos_t[0][:, 0:128], cos_t[0][:],
                                     start=(i == 0), stop=(i == n - 1))

            warmers(NWARM_GAP, "g")

            kt0 = [load_kt(0, hh) for hh in range(2)]
            kt1 = [load_kt(1, hh) for hh in range(2)]

            o_sb = big.tile([128, B * NSUB], F32, tag="o")

            def scores(b, kts):
                rs_t = [rsp.tile([128, 2 * NSUB], F32, tag=f"rs{s}",
                                 name=f"rs{b}{s}") for s in range(NSUB)]
                for hh in range(2):
                    for s in range(NSUB):
                        blocks = list(range(s + 1))
                        for c0 in range(0, len(blocks), CHUNK):
                            chunk = blocks[c0:c0 + CHUNK]
                            psl = [pps.tile([128, 512], F32, tag="ps",
                                            name=f"ps{b}{hh}{s}{c0}_{i}")
                                   for i in range(len(chunk))]
                            for dp in range(NDP):
                                lhsT = q_rope[:, 2 * dp:2 * dp + 2,
                                              RB * b + 128 * s:
                                              RB * b + 128 * (s + 1)]
                                dpl = 2 * (dp % 2)
                                for kb, ps in zip(chunk, psl):
                                    nc.tensor.matmul(
                                        ps[:], lhsT,
                                        kts[hh][dp // 2][:, dpl:dpl + 2,
                                                         kb, :, :],
                                        start=(dp == 0), stop=(dp == NDP - 1),
                                        perf_mode=DR,
                                    )
                            for kb, ps in zip(chunk, psl):
                                if kb == s:
                                    nc.vector.tensor_add(ps[:], ps[:],
                                                         mask_sb[hh][:])
                                nc.scalar.activation(
                                    ps[:], ps[:],
                                    mybir.ActivationFunctionType.Exp,
                                    scale=SCALE,
                                    accum_out=rs_t[s][:, 2 * kb + hh:
                                                      2 * kb + hh + 1],
                                )
                for s in range(NSUB):
                    nc.vector.reduce_sum(
                        o_sb[:, NSUB * b + s:NSUB * b + s + 1],
                        rs_t[s][:, 0:2 * (s + 1)], axis=mybir.AxisListType.X)
                    mlp_sub(b, s)

            def mlp_sub(b, s):
                # o_sb[p, b*NSUB+s] is local row b*RB + 128s + p.
                # z[row, n] = relu(o[row]*w1[n] + b1[n]) with o as a
                # per-partition scalar, DVE-transposed into z_aug[n, row],
                # then out rows = z_aug.T @ w2aug.
                col = NSUB * b + s
                zrow = tmp.tile([128, 32], F32, tag="zr", name=f"zr{b}{s}")
                nc.vector.tensor_scalar_mul(zrow[:], w1b_sb[:],
                                            o_sb[:, col:col + 1])
                nc.vector.tensor_add(zrow[:], zrow[:], b1b_sb[:])
                zrb = tmp.tile([128, 32], BF16, tag="zrb",
                               name=f"zrb{b}{s}")
                nc.vector.tensor_scalar_max(zrb[:], zrow[:], 0.0)
                zts = tmp.tile([32, 128], BF16, tag="zts", name=f"zts{b}{s}")
                for g in range(4):
                    nc.vector.transpose(zts[0:32, 32 * g:32 * (g + 1)],
                                        zrb[32 * g:32 * (g + 1), :])
                nc.vector.tensor_copy(z_aug[0:INNER, 128 * s:128 * (s + 1)],
                                      zts[0:INNER, :])
                row0 = RB * b + 128 * s
                ob = obp.tile([128, DIM], BF16, tag="ob", name=f"ob{b}{s}")
                for hh in range(2):
                    po = ppo.tile([128, 512], F32, tag="po",
                                  name=f"po{b}{s}{hh}")
                    nc.tensor.matmul(po[:],
                                     z_aug[:, 128 * s:128 * (s + 1)],
                                     w2_sb[:, 512 * hh:512 * (hh + 1)],
                                     start=True, stop=True)
                    nc.vector.tensor_copy(ob[:, 512 * hh:512 * (hh + 1)],
                                          po[:])
                eng = nc.gpsimd if s % 2 else nc.sync
                eng.dma_start(out_d[row0:row0 + 128, :], ob[:])

            scores(0, kt0)
            warmers(NWARM_MID, "m")
            scores(1, kt1)

    nc.compile()
    return nc


def get_nc():
    if "nc" not in _NC_CACHE:
        _NC_CACHE["nc"] = _build_nc()
    return _NC_CACHE["nc"]


def make_in_maps(hidden_states, Wq, Wk, fc1_w, fc1_b, fc2_w, fc2_b):
    hidden_states = np.asarray(hidden_states, dtype=np.float32)
    Wq = np.asarray(Wq, dtype=np.float32)
    Wk = np.asarray(Wk, dtype=np.float32)
    fc1_w = np.asarray(fc1_w, dtype=np.float32)
    fc1_b = np.asarray(fc1_b, dtype=np.float32)
    fc2_w = np.asarray(fc2_w, dtype=np.float32)
    fc2_b = np.asarray(fc2_b, dtype=np.float32)

    wqT = np.ascontiguousarray(Wq.T).astype(float8_e4m3)
    wkT = np.ascontiguousarray(Wk.T).astype(float8_e4m3)
    w1b = np.zeros((128, 32), dtype=np.float32)
    w1b[:, 0:INNER] = fc1_w.reshape(1, INNER)
    b1b = np.zeros((128, 32), dtype=np.float32)
    b1b[:, 0:INNER] = fc1_b.reshape(1, INNER)
    w2aug = np.concatenate([fc2_w.T, fc2_b[None, :]], axis=0).astype(bfloat16)

    inv_freq = ROPE_BASE ** (-np.arange(0, DIM, 2, dtype=np.float32) / DIM)

    in_maps = []
    for c in range(NCORES):
        rows = np.arange(RB) * NCORES + c            # global rows, per batch
        hT = np.concatenate(
            [hidden_states[b, rows, :].T for b in range(B)],
            axis=1).astype(float8_e4m3)              # [DIM, RLOC]
        ang = rows[:, None].astype(np.float32) * inv_freq[None, :]  # [RB,512]
        cosh = np.ascontiguousarray(np.cos(ang).T).astype(bfloat16)
        sinh = np.ascontiguousarray(np.sin(ang).T).astype(bfloat16)
        # mask_h[p, (jc-4h)*128+t]: allow k col (rank jc, t) for q row p iff
        # 8t + jc <= 8p + c  (boundary subtile; same for every s and batch)
        p = np.arange(128)[:, None, None]
        t = np.arange(128)[None, None, :]
        masks = []
        for h in range(2):
            jc = (np.arange(4) + 4 * h)[None, :, None]
            allow = (NCORES * t + jc) <= (NCORES * p + c)
            masks.append(np.where(allow, 0.0, MASK_NEG)
                         .astype(np.float32).reshape(128, 512))
        in_maps.append({
            "hT": np.ascontiguousarray(hT),
            "wqT": wqT, "wkT": wkT,
            "cosh": cosh, "sinh": sinh,
            "mask0": masks[0], "mask1": masks[1],
            "w1b": w1b, "b1b": b1b, "w2aug": w2aug,
            "onesrow": np.ones((1, RB), dtype=bfloat16),
        })
    return in_maps


def assemble_output(results):
    out = np.empty((B, L, DIM), dtype=np.float32)
    for c in range(NCORES):
        for b in range(B):
            out[b, c::NCORES, :] = (
                results[c]["out"][RB * b:RB * (b + 1)].astype(np.float32))
    return out


def run(trace=False, **inputs):
    nc = get_nc()
    in_maps = make_in_maps(**inputs)
    res = run_bass_kernel_spmd(nc, in_maps, core_ids=list(range(NCORES)),
                               trace=trace)
    return assemble_output(res.results), res


def kernel(**inputs) -> np.ndarray:
    out, _ = run(trace=False, **inputs)
    return out


# revision 17
# speedup vs baseline: 1.2676x; 1.0891x over previous
"""Distributed Trainium2 kernel for nn_AddAttention_154618823089.

Computation (see reference):
    q = rope(bf16(hidden @ Wq.T)); k = rope(bf16(hidden @ Wk.T))
    o[b,l] = sum_{j<=l} exp(q_l . k_j / sqrt(DIM))          (no softmax norm)
    out = relu(o @ fc1_w.T + fc1_b) @ fc2_w.T + fc2_b

Sharding: every core c handles the strided row set {r : r % 8 == c} of
BOTH batches (512 rows each).  Striding makes the causal workload identical
on every core, and taking rows from both batches makes the k exchange a
single fast 8-rank RDH AllGather per batch (4-rank groups fall back to the
slow Mesh path; finer splits pay a ~20us per-op cc cost).

v4 (over the 202us baseline):
  - fp8 DoubleRow pipeline as before (projections, score matmuls); exp
    fused with row-sum via accum_out; MLP per subtile
  - RoPE sin-muls moved to gpsimd so the vector queue stops lagging the
    PE by ~16us at the end of the projection phase
  - PE warmer chain (vector-paced dummy matmuls) spans the AG0 wait and
    the scores(0)->scores(1) gap so the HAM clock gate keeps the PE at
    2.4GHz instead of 1.2GHz through the scores phases
  - bf16 output (cast to f32 on host): halves output HBM traffic that
    competes with AllGather1
  - cos/sin loads halved (both batches share the same strided rows)
  - Exp activation table preloaded off the critical path
  - batch-1 kt loads stay OFF the scalar queue (scores(0) exps would be
    head-of-line blocked behind their AG1-gated DMAs)
"""

import sys
import types

import numpy as np
from ml_dtypes import bfloat16, float8_e4m3

import concourse.bacc as bacc
import concourse.bass as bass
import concourse.mybir as mybir
import concourse.tile as tile
from concourse.bass_utils import run_bass_kernel_spmd


def _install_ntff_hook():
    """The container's antenv lacks axon_hooks; provide it so trace=True can
    capture NTFF profiles (exec_time_ns) through the axon PJRT library."""
    if "antenv.axon_hooks" in sys.modules:
        return
    try:
        sys.path.insert(0, "/root/.axon_site/trn_agent_boot")
        import trn_boot

        mod = types.ModuleType("antenv.axon_hooks")
        _h = {"hook": None}
        mod.set_axon_ntff_profile_hook = lambda h: _h.__setitem__("hook", h)
        mod.get_axon_ntff_profile_hook = lambda: _h["hook"]
        sys.modules["antenv.axon_hooks"] = mod
        import antenv

        antenv.axon_hooks = mod
        mod.set_axon_ntff_profile_hook(
            trn_boot._ntff_profile_via_ctypes("/opt/axon/libaxon_pjrt.so"))
    except Exception:
        pass


_install_ntff_hook()

B, L, DIM, INNER = 2, 4096, 1024, 16
ROPE_BASE = 32.0
NCORES = 8
RB = L // NCORES       # rows per core per batch (512)
RLOC = 2 * RB          # local q/k rows per core (both batches, 1024)
NSUB = RB // 128       # q subtiles per core per batch (4)
NDT = DIM // 128       # d tiles (8)
NDP = NDT // 2         # DoubleRow d-tile pairs (4)
SCALE = 1.0 / float(np.sqrt(DIM))
MASK_NEG = -1.0e6
CHUNK = 3              # psum banks per score chunk

F32 = mybir.dt.float32
BF16 = mybir.dt.bfloat16
F8 = mybir.dt.float8e4
DR = mybir.MatmulPerfMode.DoubleRow

_NC_CACHE = {}


def _build_nc():
    nc = bacc.Bacc("TRN2", target_bir_lowering=False, debug=False,
                   num_devices=NCORES, num_swdge_queues=4)

    hT = nc.dram_tensor("hT", [DIM, RLOC], F8, kind="ExternalInput")
    wqT = nc.dram_tensor("wqT", [DIM, DIM], F8, kind="ExternalInput")
    wkT = nc.dram_tensor("wkT", [DIM, DIM], F8, kind="ExternalInput")
    cosh = nc.dram_tensor("cosh", [DIM // 2, RB], BF16, kind="ExternalInput")
    sinh = nc.dram_tensor("sinh", [DIM // 2, RB], BF16, kind="ExternalInput")
    mask0 = nc.dram_tensor("mask0", [128, 512], F32, kind="ExternalInput")
    mask1 = nc.dram_tensor("mask1", [128, 512], F32, kind="ExternalInput")
    w1b_d = nc.dram_tensor("w1b", [128, 32], F32, kind="ExternalInput")
    b1b_d = nc.dram_tensor("b1b", [128, 32], F32, kind="ExternalInput")
    w2aug = nc.dram_tensor("w2aug", [INNER + 1, DIM], BF16, kind="ExternalInput")
    onesrow = nc.dram_tensor("onesrow", [1, RB], BF16, kind="ExternalInput")
    out_d = nc.dram_tensor("out", [RLOC, DIM], BF16, kind="ExternalOutput")

    # one bounce + AllGather per batch: small collectives pay a ~20us
    # fixed per-op cost on the cc stream, so two 4MB-out gathers beat any
    # finer split
    kb_bounce = [nc.dram_tensor(f"kTb{b}", [128, NDT, RB], F8)
                 for b in range(B)]
    G = [nc.dram_tensor(f"G{b}", [NCORES * 128, NDT, RB], F8,
                        addr_space="Shared") for b in range(B)]

    groups = [list(range(NCORES))]

    with tile.TileContext(nc) as tc:
        with (
            tc.tile_pool(name="big", bufs=1) as big,
            tc.tile_pool(name="tmp", bufs=2) as tmp,
            tc.tile_pool(name="stg", bufs=2) as stg,
            tc.tile_pool(name="rsp", bufs=2) as rsp,
            tc.tile_pool(name="obp", bufs=4) as obp,
            tc.tile_pool(name="ps", bufs=7, space="PSUM") as pps,
            tc.tile_pool(name="po", bufs=1, space="PSUM") as ppo,
        ):
            # ---- inputs -> SBUF as DoubleRow pair tiles, spread on queues --
            # (dp p k2 r) views land each pair tile in ONE dma each
            h_r = hT.rearrange("(dp k2 p) r -> dp p k2 r", dp=NDP, k2=2, p=128)
            wk_r = wkT.rearrange("(dp k2 p) r -> dp p k2 r",
                                 dp=NDP, k2=2, p=128)
            wq_r = wqT.rearrange("(dp k2 p) r -> dp p k2 r",
                                 dp=NDP, k2=2, p=128)
            h_t, wk_t, wq_t = [], [], []
            for dp in range(NDP):
                th = big.tile([128, 2, RLOC], F8, tag=f"h{dp}", name=f"h{dp}")
                nc.sync.dma_start(th[:], h_r[dp])
                h_t.append(th)
                tw = big.tile([128, 2, DIM], F8, tag=f"wk{dp}", name=f"wk{dp}")
                nc.scalar.dma_start(tw[:], wk_r[dp])
                wk_t.append(tw)
            cos_t, sin_t = [], []
            for ci in range(NDT // 2):
                tc_ = big.tile([128, RB], BF16, tag=f"cos{ci}",
                               name=f"cos{ci}")
                nc.sync.dma_start(tc_[:], cosh[128 * ci:128 * (ci + 1), :])
                cos_t.append(tc_)
                ts_ = big.tile([128, RB], BF16, tag=f"sin{ci}",
                               name=f"sin{ci}")
                nc.scalar.dma_start(ts_[:], sinh[128 * ci:128 * (ci + 1), :])
                sin_t.append(ts_)
            for dp in range(NDP):
                # wq reuses wk's slots (k projection is done by then)
                tw = big.tile([128, 2, DIM], F8, tag=f"wk{dp}", name=f"wq{dp}")
                nc.sync.dma_start(tw[:], wq_r[dp])
                wq_t.append(tw)
            mask_sb = [big.tile([128, 512], F32, tag=f"mask{h}",
                                name=f"mask_sb{h}") for h in range(2)]
            nc.gpsimd.dma_start(mask_sb[0][:], mask0[:])
            nc.gpsimd.dma_start(mask_sb[1][:], mask1[:])
            w1b_sb = big.tile([128, 32], F32, tag="w1b")
            nc.gpsimd.dma_start(w1b_sb[:], w1b_d[:])
            b1b_sb = big.tile([128, 32], F32, tag="b1b")
            nc.gpsimd.dma_start(b1b_sb[:], b1b_d[:])
            w2_sb = big.tile([INNER + 1, DIM], BF16, tag="w2")
            nc.gpsimd.dma_start(w2_sb[:], w2aug[:])
            z_aug = big.tile([INNER + 1, RB], BF16, tag="zaug")
            nc.gpsimd.dma_start(z_aug[INNER:INNER + 1, :], onesrow[:])

            # pre-load the Exp activation table off the critical path
            etab = tmp.tile([1, 4], F32, tag="etab", name="etab")
            nc.scalar.activation(etab[:], w1b_sb[0:1, 0:4],
                                 mybir.ActivationFunctionType.Exp)

            def project_half(w_t, proj, rt, bounce=False):
                """proj[:, :, 512rt:512rt+512] = fp8(rope(W @ h^T)).
                DoubleRow fp8 matmuls -> psum f32 -> bf16 staging (scalar)
                -> rope on vector+gpsimd -> fp8 slots (dt, dt+4); do-order
                interleaves the (dt, dt+4) halves so RoPE pairs complete
                (and optionally bounce to DRAM) right behind PE."""
                cols = slice(512 * rt, 512 * (rt + 1))
                pbf = stg.tile([128, NDT, 512], BF16, tag="pbf",
                               name=f"pbf{rt}")

                def rope_pair(dt):
                    # both batches share the same strided rows, so one
                    # cos/sin tile serves rt=0 and rt=1
                    cm = cos_t[dt][:, :]
                    sm = sin_t[dt][:, :]
                    lo = pbf[:, dt, :]
                    hi = pbf[:, dt + NDT // 2, :]
                    ta = tmp.tile([128, 512], BF16, tag="ta", name="ta")
                    tb = tmp.tile([128, 512], BF16, tag="tb", name="tb")
                    td = tmp.tile([128, 512], BF16, tag="td", name="td")
                    # all on vector: gpsimd elementwise is 3x slower AND
                    # anything queued on gpsimd behind a collective trigger
                    # blocks until the cc stream accepts that trigger
                    nc.vector.tensor_mul(ta[:], lo, cm)
                    nc.vector.tensor_mul(tb[:], lo, sm)
                    nc.vector.tensor_mul(td[:], hi, sm)
                    nc.vector.tensor_sub(proj[:, dt, cols], ta[:], td[:])
                    nc.vector.tensor_mul(ta[:], hi, cm)
                    nc.vector.tensor_add(proj[:, dt + NDT // 2, cols],
                                         ta[:], tb[:])
                    if bounce:
                        # both rope slots of the pair in one strided dma
                        eng = nc.sync if dt % 2 else nc.scalar
                        eng.dma_start(
                            kb_bounce[rt][:, dt::NDT // 2, :],
                            proj[:, dt::NDT // 2, cols])

                order = [x for pair in zip(range(NDT // 2),
                                           range(NDT // 2, NDT))
                         for x in pair]            # 0,4,1,5,2,6,3,7
                for do in order:
                    ps = pps.tile([128, 512], F32, tag="ps",
                                  name=f"psp{rt}{do}")
                    for dp in range(NDP):
                        nc.tensor.matmul(
                            ps[:], w_t[dp][:, :, 128 * do:128 * (do + 1)],
                            h_t[dp][:, :, cols],
                            start=(dp == 0), stop=(dp == NDP - 1),
                            perf_mode=DR,
                        )
                    # f32 psum -> bf16 staging for rope (reference casts
                    # q/k to bf16 here); scalar ACT keeps vector free for
                    # rope and unblocks psum banks for the next matmuls
                    nc.scalar.activation(pbf[:, do, :], ps[:],
                                         mybir.ActivationFunctionType.Copy)
                    if do >= NDT // 2:
                        rope_pair(do - NDT // 2)

            # ---- gathered-K load helper ------------------------------------
            g_r = [G[b].rearrange("(r p) t (kb jj) -> r p t kb jj",
                                  r=NCORES, p=128, kb=NSUB, jj=128)
                   for b in range(B)]
            # batch-1 loads stay off the scalar queue: scores(0)'s exps
            # would otherwise be head-of-line blocked behind AG1-gated dmas
            _kteng = {0: [nc.sync, nc.scalar, nc.gpsimd],
                      1: [nc.sync, nc.gpsimd]}

            def load_kt(b, hh):
                # kt layout: [128 (d in tile), t4, kb4, r4, jj128] with a
                # SEPARATE tile per t-half, so score matmuls on dp 0-1
                # start as soon as the first half of the transfer lands;
                # the DoubleRow moving slice [:, 2dp':2dp'+2, kb, :, :]
                # flattens to [128, 2, 512] (kb-major puts the block's
                # (r, jj) columns contiguous in SBUF).
                engs = _kteng[b]
                kts, i = [], hh
                for dh in range(2):
                    kt = big.tile([128, NDT // 2, NSUB, 4, 128], F8,
                                  tag=f"kt{b}{hh}{dh}", name=f"kt{b}{hh}{dh}")
                    for r in range(4):
                        eng = engs[i % len(engs)]
                        i += 1
                        eng.dma_start(
                            kt[:, :, :, r, :],
                            g_r[b][4 * hh + r, :, 4 * dh:4 * (dh + 1), :, :])
                    kts.append(kt)
                return kts

            # ---- k per batch: project+rope+bounce, then both all-gathers
            # back to back on the cc stream; q projects during the
            # collectives; kt loads are emitted last so no engine stream
            # has compute queued behind a gather-gated dma issue ---------
            k_rope = big.tile([128, NDT, RLOC], F8, tag="krope")
            project_half(wk_t, k_rope, 0, bounce=True)
            nc.gpsimd.collective_compute(
                "AllGather", mybir.AluOpType.bypass, replica_groups=groups,
                ins=[kb_bounce[0].ap().opt()], outs=[G[0].ap().opt()])
            project_half(wk_t, k_rope, 1, bounce=True)
            nc.gpsimd.collective_compute(
                "AllGather", mybir.AluOpType.bypass, replica_groups=groups,
                ins=[kb_bounce[1].ap().opt()], outs=[G[1].ap().opt()])

            # ---- q: project + rope (overlaps with the collectives) ----
            q_rope = big.tile([128, NDT, RLOC], F8, tag="qrope")
            project_half(wq_t, q_rope, 0)
            project_half(wq_t, q_rope, 1)

            kt0 = [load_kt(0, hh) for hh in range(2)]
            kt1 = [load_kt(1, hh) for hh in range(2)]

            o_sb = big.tile([128, B * NSUB], F32, tag="o")

            def scores(b, kts):
                rs_t = [rsp.tile([128, 2 * NSUB], F32, tag=f"rs{s}",
                                 name=f"rs{b}{s}") for s in range(NSUB)]
                for hh in range(2):
                    for s in range(NSUB):
                        blocks = list(range(s + 1))
                        for c0 in range(0, len(blocks), CHUNK):
                            chunk = blocks[c0:c0 + CHUNK]
                            psl = [pps.tile([128, 512], F32, tag="ps",
                                            name=f"ps{b}{hh}{s}{c0}_{i}")
                                   for i in range(len(chunk))]
                            for dp in range(NDP):
                                lhsT = q_rope[:, 2 * dp:2 * dp + 2,
                                              RB * b + 128 * s:
                                              RB * b + 128 * (s + 1)]
                                dpl = 2 * (dp % 2)
                                for kb, ps in zip(chunk, psl):
                                    nc.tensor.matmul(
                                        ps[:], lhsT,
                                        kts[hh][dp // 2][:, dpl:dpl + 2,
                                                         kb, :, :],
                                        start=(dp == 0), stop=(dp == NDP - 1),
                                        perf_mode=DR,
                                    )
                            for kb, ps in zip(chunk, psl):
                                if kb == s:
                                    nc.vector.tensor_add(ps[:], ps[:],
                                                         mask_sb[hh][:])
                                nc.scalar.activation(
                                    ps[:], ps[:],
                                    mybir.ActivationFunctionType.Exp,
                                    scale=SCALE,
                                    accum_out=rs_t[s][:, 2 * kb + hh:
                                                      2 * kb + hh + 1],
                                )
                for s in range(NSUB):
                    nc.vector.reduce_sum(
                        o_sb[:, NSUB * b + s:NSUB * b + s + 1],
                        rs_t[s][:, 0:2 * (s + 1)], axis=mybir.AxisListType.X)
                    mlp_sub(b, s)

            def mlp_sub(b, s):
                # o_sb[p, b*NSUB+s] is local row b*RB + 128s + p.
                # z[row, n] = relu(o[row]*w1[n] + b1[n]) with o as a
                # per-partition scalar, DVE-transposed into z_aug[n, row],
                # then out rows = z_aug.T @ w2aug.
                col = NSUB * b + s
                zrow = tmp.tile([128, 32], F32, tag="zr", name=f"zr{b}{s}")
                nc.vector.tensor_scalar_mul(zrow[:], w1b_sb[:],
                                            o_sb[:, col:col + 1])
                nc.vector.tensor_add(zrow[:], zrow[:], b1b_sb[:])
                zrb = tmp.tile([128, 32], BF16, tag="zrb",
                               name=f"zrb{b}{s}")
                nc.vector.tensor_scalar_max(zrb[:], zrow[:], 0.0)
                zts = tmp.tile([32, 128], BF16, tag="zts", name=f"zts{b}{s}")
                for g in range(4):
                    nc.vector.transpose(zts[0:32, 32 * g:32 * (g + 1)],
                                        zrb[32 * g:32 * (g + 1), :])
                nc.vector.tensor_copy(z_aug[0:INNER, 128 * s:128 * (s + 1)],
                                      zts[0:INNER, :])
                row0 = RB * b + 128 * s
                ob = obp.tile([128, DIM], BF16, tag="ob", name=f"ob{b}{s}")
                for hh in range(2):
                    po = ppo.tile([128, 512], F32, tag="po",
                                  name=f"po{b}{s}{hh}")
                    nc.tensor.matmul(po[:],
                                     z_aug[:, 128 * s:128 * (s + 1)],
                                     w2_sb[:, 512 * hh:512 * (hh + 1)],
                                     start=True, stop=True)
                    nc.vector.tensor_copy(ob[:, 512 * hh:512 * (hh + 1)],
                                          po[:])
                eng = nc.gpsimd if s % 2 else nc.sync
                eng.dma_start(out_d[row0:row0 + 128, :], ob[:])

            scores(0, kt0)
            scores(1, kt1)

    nc.compile()
    return nc


def get_nc():
    if "nc" not in _NC_CACHE:
        _NC_CACHE["nc"] = _build_nc()
    return _NC_CACHE["nc"]


def make_in_maps(hidden_states, Wq, Wk, fc1_w, fc1_b, fc2_w, fc2_b):
    hidden_states = np.asarray(hidden_states, dtype=np.float32)
    Wq = np.asarray(Wq, dtype=np.float32)
    Wk = np.asarray(Wk, dtype=np.float32)
    fc1_w = np.asarray(fc1_w, dtype=np.float32)
    fc1_b = np.asarray(fc1_b, dtype=np.float32)
    fc2_w = np.asarray(fc2_w, dtype=np.float32)
    fc2_b = np.asarray(fc2_b, dtype=np.float32)

    wqT = np.ascontiguousarray(Wq.T).astype(float8_e4m3)
    wkT = np.ascontiguousarray(Wk.T).astype(float8_e4m3)
    w1b = np.zeros((128, 32), dtype=np.float32)
    w1b[:, 0:INNER] = fc1_w.reshape(1, INNER)
    b1b = np.zeros((128, 32), dtype=np.float32)
    b1b[:, 0:INNER] = fc1_b.reshape(1, INNER)
    w2aug = np.concatenate([fc2_w.T, fc2_b[None, :]], axis=0).astype(bfloat16)

    inv_freq = ROPE_BASE ** (-np.arange(0, DIM, 2, dtype=np.float32) / DIM)

    in_maps = []
    for c in range(NCORES):
        rows = np.arange(RB) * NCORES + c            # global rows, per batch
        hT = np.concatenate(
            [hidden_states[b, rows, :].T for b in range(B)],
            axis=1).astype(float8_e4m3)              # [DIM, RLOC]
        ang = rows[:, None].astype(np.float32) * inv_freq[None, :]  # [RB,512]
        cosh = np.ascontiguousarray(np.cos(ang).T).astype(bfloat16)
        sinh = np.ascontiguousarray(np.sin(ang).T).astype(bfloat16)
        # mask_h[p, (jc-4h)*128+t]: allow k col (rank jc, t) for q row p iff
        # 8t + jc <= 8p + c  (boundary subtile; same for every s and batch)
        p = np.arange(128)[:, None, None]
        t = np.arange(128)[None, None, :]
        masks = []
        for h in range(2):
            jc = (np.arange(4) + 4 * h)[None, :, None]
            allow = (NCORES * t + jc) <= (NCORES * p + c)
            masks.append(np.where(allow, 0.0, MASK_NEG)
                         .astype(np.float32).reshape(128, 512))
        in_maps.append({
            "hT": np.ascontiguousarray(hT),
            "wqT": wqT, "wkT": wkT,
            "cosh": cosh, "sinh": sinh,
            "mask0": masks[0], "mask1": masks[1],
            "w1b": w1b, "b1b": b1b, "w2aug": w2aug,
            "onesrow": np.ones((1, RB), dtype=bfloat16),
        })
    return in_maps


def assemble_output(results):
    out = np.empty((B, L, DIM), dtype=np.float32)
    for c in range(NCORES):
        for b in range(B):
            out[b, c::NCORES, :] = (
                results[c]["out"][RB * b:RB * (b + 1)].astype(np.float32))
    return out


def run(trace=False, **inputs):
    nc = get_nc()
    in_maps = make_in_maps(**inputs)
    res = run_bass_kernel_spmd(nc, in_maps, core_ids=list(range(NCORES)),
                               trace=trace)
    return assemble_output(res.results), res


def kernel(**inputs) -> np.ndarray:
    out, _ = run(trace=False, **inputs)
    return out
